# revision 36
# baseline (speedup 1.0000x reference)
"""Trainium2 Bass kernel for a dense transformer block (B=2, N=1024, D=768,
H=12, MLP=3072) returning (x_out, attn_mean, uncertainty).

Sharding: 8-way row-parallel. Core c handles batch b=c//4 and token rows
rb*256:(rb+1)*256 (rb=c%4). Each core redundantly computes LN1 + k/v over
the full sequence of its batch, so there are no collectives. Inputs are fed
with a per-core block-roll of the token axis so "our" rows are always rows
0:256 in the on-device view; the host un-permutes the key axis of the
attention outputs when gathering.

Head-mixing 1x1 conv trick: uncertainty_g = sigmoid(q~_g^T @ k + conv_b[g])
where q~_g[h*64+d, n] = conv_w[g, h] * qT[h*64+d, n] — a full K=768 matmul
instead of a cross-partition head reduction.
"""

import sys

if "/opt/trn_rl_repo" not in sys.path:
    sys.path.insert(0, "/opt/trn_rl_repo")

import ml_dtypes
import numpy as np
from contextlib import ExitStack

import concourse.bass as bass
import concourse.bacc as bacc
import concourse.mybir as mybir
from concourse import masks, tile
from concourse.bass_utils import run_bass_kernel_spmd

DIM, HEADS, MLP, B, N = 768, 12, 3072, 2, 1024
HD = DIM // HEADS            # 64
SCALE = float(HD) ** -0.5
NCORES = 8
RB = NCORES // B             # 4 row-blocks per batch
R = N // RB                  # 256 rows per core
NT = R // 128                # 2 n-tiles per core
TT = N // 128                # 8 token tiles (full sequence)
DT = DIM // 128              # 6 d-tiles
MT = MLP // 128              # 24 mlp tiles
EPS = 1e-5

F32 = mybir.dt.float32
BF16 = mybir.dt.bfloat16
F32R = mybir.dt.float32r
ALU = mybir.AluOpType
AFT = mybir.ActivationFunctionType
AX = mybir.AxisListType

LAST_RESULTS = None  # BassKernelResults stash for test harness introspection
LAST_PROGRAM = None  # (nc, in_maps) stash for test-harness benchmarking


def r32(ap):
    """Reinterpret an fp32 AP as float32r for full-rate PE matmuls."""
    return ap.bitcast(F32R)


def _build_program():
    nc = bacc.Bacc("TRN2", target_bir_lowering=False, debug=False,
                   num_devices=NCORES)

    # ---- DRAM I/O ----
    xb = nc.dram_tensor("xb", [N, DIM], F32, kind="ExternalInput").ap()
    rs = nc.dram_tensor("rs", [HEADS, N, R], F32, kind="ExternalInput").ap()
    w_qkv = nc.dram_tensor("w_qkv", [DIM, 3 * DIM], F32R, kind="ExternalInput").ap()
    w_proj = nc.dram_tensor("w_proj", [DIM, DIM], BF16, kind="ExternalInput").ap()
    w_fc1 = nc.dram_tensor("w_fc1", [DIM, MLP], BF16, kind="ExternalInput").ap()
    w_fc2 = nc.dram_tensor("w_fc2", [MLP, DIM], BF16, kind="ExternalInput").ap()
    qkb_colt = nc.dram_tensor("qkb_colt", [128, 2 * DIM // 128], F32,
                              kind="ExternalInput").ap()
    fc1b_colt = nc.dram_tensor("fc1b_colt", [128, MLP // 128], F32,
                               kind="ExternalInput").ap()
    cexp = nc.dram_tensor("cexp", [DIM, HEADS], F32, kind="ExternalInput").ap()
    bc7 = nc.dram_tensor("bc7", [7, 128, DIM], F32, kind="ExternalInput").ap()
    cb_bc = nc.dram_tensor("cb_bc", [128, HEADS], F32, kind="ExternalInput").ap()
    onesrr = nc.dram_tensor("onesrr", [2, 128], F32R, kind="ExternalInput").ap()

    out_x = nc.dram_tensor("out_x", [R, DIM], F32, kind="ExternalOutput").ap()
    out_am = nc.dram_tensor("out_am", [HEADS, N, R], F32, kind="ExternalOutput").ap()
    out_unc = nc.dram_tensor("out_unc", [HEADS, N, R], F32, kind="ExternalOutput").ap()

    with tile.TileContext(nc, num_cores=NCORES) as tc, ExitStack() as ctx:
        _emit(ctx, tc, nc, dict(
            xb=xb, rs=rs, w_qkv=w_qkv, w_proj=w_proj,
            w_fc1=w_fc1, w_fc2=w_fc2, qkb_colt=qkb_colt, fc1b_colt=fc1b_colt,
            cexp=cexp, bc7=bc7, cb_bc=cb_bc, onesrr=onesrr, out_x=out_x, out_am=out_am, out_unc=out_unc,
        ))
    nc.compile()
    return nc


def _bcast_load(nc, pool, dram_plane, n, tag):
    """DRAM [128, n] host-prebroadcast plane -> SBUF [128, n] tile."""
    out = pool.tile([128, n], F32, tag=tag, name=tag)
    nc.sync.dma_start(out[:, :], dram_plane)
    return out


def _col_plane(nc, pool, dram_plane, ncols, tag):
    """DRAM [128, ncols] host-preshaped bias-column plane -> SBUF tile.
    Column j holds bias[j*128:(j+1)*128]."""
    t = pool.tile([128, ncols], F32, tag=tag, name=tag)
    nc.sync.dma_start(t[:, :], dram_plane)
    return t


def _layernorm(nc, scratch, xt, g_b, b_b, out_t, idx, eps_col):
    """LN over free dim (768) of [128, 768] tile xt -> out_t."""
    mu = scratch.tile([128, 1], F32, tag="ln_mu", name=f"mu{idx}")
    nc.vector.reduce_sum(mu[:, :], xt[:, :], axis=AX.X)
    nc.vector.tensor_scalar_mul(mu[:, :], mu[:, :], 1.0 / DIM)
    # out = x - mu
    nc.vector.tensor_scalar_sub(out_t[:, :], xt[:, :], mu[:, :])
    sq = scratch.tile([128, DIM], F32, tag="ln_sq", name=f"sq{idx}")
    ssq = scratch.tile([128, 1], F32, tag="ln_ssq", name=f"ssq{idx}")
    nc.vector.scalar_tensor_tensor(sq[:, :], out_t[:, :], 1.0, out_t[:, :],
                                   op0=ALU.mult, op1=ALU.mult,
                                   accum_out=ssq[:, :])
    std = scratch.tile([128, 1], F32, tag="ln_std", name=f"std{idx}")
    # std = sqrt(ssq/DIM + eps)
    nc.scalar.activation(std[:, :], ssq[:, :], AFT.Sqrt, bias=eps_col[:, :],
                         scale=1.0 / DIM)
    rstd = scratch.tile([128, 1], F32, tag="ln_rstd", name=f"rstd{idx}")
    nc.vector.reciprocal(rstd[:, :], std[:, :])
    # out = (out * rstd) * g ; out += b
    nc.vector.scalar_tensor_tensor(out_t[:, :], out_t[:, :], rstd[:, :],
                                   g_b[:, :], op0=ALU.mult, op1=ALU.mult)
    nc.vector.tensor_add(out_t[:, :], out_t[:, :], b_b[:, :])


def _emit(ctx, tc, nc, io):
    xb, rs = io["xb"], io["rs"]

    # ---------- persistent pools ----------
    const = ctx.enter_context(tc.tile_pool(name="const", bufs=1))
    persist = ctx.enter_context(tc.tile_pool(name="persist", bufs=1))

    # x rows 0:256 (ours) first — LN1 is the head of the critical path
    xres = []
    for t in range(NT):
        xt = persist.tile([128, DIM], F32, tag=f"xres{t}", name=f"xres{t}")
        nc.sync.dma_start(xt[:, :], xb[t * 128:(t + 1) * 128, :])
        xres.append(xt)

    identity = const.tile([128, 128], F32, tag="ident", name="ident")
    masks.make_identity(nc, identity[:, :])
    eps_col = const.tile([128, 1], F32, tag="eps", name="eps")
    nc.gpsimd.memset(eps_col[:, :], EPS)
    bc7 = io["bc7"]
    cb_b = _bcast_load(nc, const, io["cb_bc"], HEADS, "cb")
    qkb_plane = _col_plane(nc, const, io["qkb_colt"], 2 * DIM // 128, "qkb")
    fc1b_plane = _col_plane(nc, const, io["fc1b_colt"], MLP // 128, "f1b")

    cexp_t = []
    for j in range(DT):
        t = const.tile([128, HEADS], F32, tag=f"cexp{j}", name=f"cexp{j}")
        nc.sync.dma_start(t[:, :], io["cexp"][j * 128:(j + 1) * 128, :])
        cexp_t.append(t)

    # qT [768, 256] and attn-out^T [768, 256], alive through attention
    qT = [persist.tile([128, R], F32R, tag=f"qT{j}", name=f"qT{j}")
          for j in range(DT)]
    aoT = [persist.tile([128, R], BF16, tag=f"aoT{j}", name=f"aoT{j}")
           for j in range(DT)]

    # ---------- phase 1+2: LN1 over full batch, h -> hT ----------
    # Pool stacking: left stack holds const/persist -> pool_h (closed after
    # qkv) -> attention scratch -> mlp. kT/v live on the right stack since
    # their lifetime (phases 3-4) straddles pool_h's close.
    es_h = ctx.enter_context(ExitStack())
    pool_h = es_h.enter_context(tc.tile_pool(name="pool_h", bufs=1))
    ln1g_b = _bcast_load(nc, pool_h, bc7[0], DIM, "ln1g")
    ln1b_b = _bcast_load(nc, pool_h, bc7[1], DIM, "ln1b")
    vb_b = _bcast_load(nc, pool_h, bc7[6], DIM, "vb")
    hT = [pool_h.tile([128, N], F32R, tag=f"hT{j}", name=f"hT{j}")
          for j in range(DT)]
    es_kv = ctx.enter_context(ExitStack())
    kv = es_kv.enter_context(tc.tile_pool(name="kv", bufs=1, side="right"))
    kT = [kv.tile([128, N], F32R, tag=f"kT{j}", name=f"kT{j}")
          for j in range(DT)]
    vtok = [kv.tile([128, DIM], F32R, tag=f"v{t}", name=f"v{t}")
            for t in range(TT)]

    with tc.tile_pool(name="ln1_scr", bufs=2) as scr, \
         tc.tile_pool(name="ps_t1", bufs=4, space="PSUM") as ps_t1:
        for t in range(TT):
            if t < NT:
                xt = xres[t]
            else:
                xt = scr.tile([128, DIM], F32, tag="x_full", name=f"x{t}")
                nc.sync.dma_start(xt[:, :], xb[t * 128:(t + 1) * 128, :])
            ht = scr.tile([128, DIM], F32, tag="h_full", name=f"h{t}")
            _layernorm(nc, scr, xt, ln1g_b, ln1b_b, ht, t, eps_col)
            for j in range(DT):
                pt = ps_t1.tile([128, 128], F32, tag="tp", name=f"tp{t}_{j}")
                nc.tensor.transpose(pt[:, :], ht[:, j * 128:(j + 1) * 128],
                                    identity[:, :])
                nc.scalar.copy(hT[j][:, t * 128:(t + 1) * 128], pt[:, :])

    # ---------- phase 3: qkv ----------
    with tc.tile_pool(name="wqkv", bufs=1, side="right") as wq_pool, \
         tc.tile_pool(name="ps_qkv", bufs=2, space="PSUM") as ps_qkv:
        wqkv = []
        for j in range(DT):
            wt = wq_pool.tile([128, 3 * DIM], F32R, tag=f"wqkv{j}",
                              name=f"wqkv{j}")
            nc.sync.dma_start(wt[:, :], io["w_qkv"][j * 128:(j + 1) * 128, :])
            wqkv.append(wt)

        # q (feature-major, our 256 rows): out [f128, 256]
        for j in range(DT):
            pq = ps_qkv.tile([128, R], F32, tag="ps_q", name=f"psq{j}",
                             bufs=1)
            for dk in range(DT):
                nc.tensor.matmul(pq[:, :],
                                 r32(wqkv[dk][:, j * 128:(j + 1) * 128]),
                                 r32(hT[dk][:, 0:R]),
                                 start=(dk == 0), stop=(dk == DT - 1))
            nc.scalar.add(qT[j][:, :], pq[:, :], qkb_plane[:, j:j + 1])

        # k (feature-major, full sequence): out [f128, 1024]
        for j in range(DT):
            pk = ps_qkv.tile([128, N], F32, tag="ps_k", name=f"psk{j}")
            for mb in range(2):
                sl = slice(mb * 512, (mb + 1) * 512)
                for dk in range(DT):
                    nc.tensor.matmul(
                        pk[:, sl],
                        r32(wqkv[dk][:, DIM + j * 128:DIM + (j + 1) * 128]),
                        r32(hT[dk][:, sl]),
                        start=(dk == 0), stop=(dk == DT - 1))
            nc.scalar.add(kT[j][:, :], pk[:, :],
                          qkb_plane[:, DT + j:DT + j + 1])

        # v (token-major, full sequence): out [tok128, 768]
        for t in range(TT):
            pv = ps_qkv.tile([128, DIM], F32, tag="ps_v", name=f"psv{t}",
                             bufs=1)
            for fb, fw in ((0, 512), (512, 256)):
                for dk in range(DT):
                    nc.tensor.matmul(
                        pv[:, fb:fb + fw],
                        r32(hT[dk][:, t * 128:(t + 1) * 128]),
                        r32(wqkv[dk][:, 2 * DIM + fb:2 * DIM + fb + fw]),
                        start=(dk == 0), stop=(dk == DT - 1))
            nc.vector.tensor_add(vtok[t][:, :], pv[:, :], vb_b[:, :])

    es_h.close()  # frees hT

    # ---------- MLP weight preload (hoistable into attention DMA idle) ----
    fcw = ctx.enter_context(tc.tile_pool(name="fc_w", bufs=1))
    wfc1 = []
    for dk in range(DT):
        wt = fcw.tile([128, MLP], BF16, tag=f"wfc1_{dk}", name=f"wfc1_{dk}")
        nc.sync.dma_start(wt[:, :], io["w_fc1"][dk * 128:(dk + 1) * 128, :])
        wfc1.append(wt)
    g1 = [fcw.tile([128, R], BF16, tag=f"g1_{ft}", name=f"g1_{ft}")
          for ft in range(MT)]

    # ---------- phase 4: attention ----------
    # Everything in [m, n] (transposed) layout. Per head: qk^T -> exp
    # (unnormalized) -> column sums via a ones-column matmul (cross-partition
    # reduce on PE) -> invB = 1/sums broadcast via rank-1 matmul ->
    # attn_mean^T = exp * invB; unc^T = sigmoid(k^T-contraction with the
    # conv-scaled q); attn^T = attn_mean^T + unc^T * r^T feeds av directly.
    # ACT runs only Exp and Sigmoid, grouped, so table reloads stay rare.
    ones_col = const.tile([1, 128], F32R, tag="ones_col", name="ones_col")
    nc.sync.dma_start(ones_col[:, :], io["onesrr"][0:1, :])
    ones128 = const.tile([128, 1], F32R, tag="ones128", name="ones128")
    nc.sync.dma_start(ones128[:, :],
                      io["onesrr"][1].rearrange("(p o) -> p o", o=1))
    with tc.tile_pool(name="at_sb", bufs=2) as asb, \
         tc.tile_pool(name="at_qg", bufs=2) as qg_pool, \
         tc.tile_pool(name="at_ps", bufs=2, space="PSUM") as ps2:
        for h in range(HEADS):
            jj, po = divmod(h, 2)
            po *= 64
            # q~_g = qT * cexp[:, h]  (for the uncertainty matmul)
            qg = [qg_pool.tile([128, R], F32R, tag=f"qg{j}", name=f"qg{h}_{j}")
                  for j in range(DT)]
            for j in range(DT):
                nc.vector.tensor_scalar_mul(qg[j][:, :],
                                            qT[j][:, :].bitcast(F32),
                                            cexp_t[j][:, h:h + 1])

            # exp(SCALE * qk^T), unnormalized, by m-pair
            exs = []
            for mp in range(TT // 2):
                pT = ps2.tile([128, 512], F32, tag="ps_qkt",
                              name=f"pqkt{h}_{mp}")
                for half in range(2):
                    m = 2 * mp + half
                    msl = slice(m * 128, (m + 1) * 128)
                    nc.tensor.matmul(pT[:, half * R:(half + 1) * R],
                                     kT[jj][po:po + 64, msl],
                                     qT[jj][po:po + 64, 0:R],
                                     start=True, stop=True)
                ex = asb.tile([128, 512], F32R, tag=f"ex{mp}",
                              name=f"ex{h}_{mp}", bufs=2)
                nc.scalar.activation(ex[:, :], pT[:, :], AFT.Exp, scale=SCALE)
                exs.append(ex)
            # column sums -> inv -> invB broadcast
            pss = ps2.tile([128, 512], F32, tag="ps_small", name=f"pss{h}",
                           bufs=1)
            for mp in range(TT // 2):
                for half in range(2):
                    nc.tensor.matmul(pss[0:1, 0:R], ones128[:, :],
                                     exs[mp][:, half * R:(half + 1) * R],
                                     start=(mp == 0 and half == 0),
                                     stop=(mp == TT // 2 - 1 and half == 1))
            invrow = asb.tile([1, R], F32R, tag="invrow", name=f"ivr{h}")
            with nc.allow_low_precision(reason="f32r rounding for PE rank-1"):
                nc.vector.reciprocal(invrow[:, :], pss[0:1, 0:R])
            psb = ps2.tile([128, 512], F32, tag="ps_small", name=f"psb{h}",
                           bufs=1)
            for half in range(2):
                nc.tensor.matmul(psb[:, half * R:(half + 1) * R],
                                 ones_col[:, :], invrow[:, :],
                                 start=True, stop=True)
            invB = asb.tile([128, 512], F32, tag="invB", name=f"invB{h}")
            nc.vector.tensor_copy(invB[:, :], psb[:, :])

            # attn_mean^T = exp * invB
            amt = []
            for mp in range(TT // 2):
                a = asb.tile([128, 512], F32, tag=f"amt{mp}",
                             name=f"amt{h}_{mp}", bufs=2)
                nc.vector.tensor_mul(a[:, :], exs[mp][:, :].bitcast(F32),
                                     invB[:, :])
                nc.sync.dma_start(
                    io["out_am"][h].rearrange("(mp p) n -> p mp n", p=128)[:, 2 * mp:2 * mp + 2, :],
                    a[:, :].rearrange("p (a n) -> p a n", a=2))
                amt.append(a)

            # unc^T
            utps = []
            rtps = []
            for mp in range(TT // 2):
                rtp = asb.tile([128, 512], F32, tag=f"rt{mp}",
                               name=f"rt{h}_{mp}", bufs=3)
                nc.sync.dma_start(
                    rtp[:, :].rearrange("p (a n) -> p a n", a=2),
                    rs[h].rearrange("(mp p) n -> p mp n", p=128)[:, 2 * mp:2 * mp + 2, :])
                pU = ps2.tile([128, 512], F32, tag="ps_unc",
                              name=f"pun{h}_{mp}")
                for half in range(2):
                    m = 2 * mp + half
                    msl = slice(m * 128, (m + 1) * 128)
                    for dk in range(DT):
                        nc.tensor.matmul(pU[:, half * R:(half + 1) * R],
                                         kT[dk][:, msl],
                                         qg[dk][:, 0:R],
                                         start=(dk == 0), stop=(dk == DT - 1))
                utp = asb.tile([128, 512], F32, tag=f"unc{mp}",
                               name=f"ut{h}_{mp}", bufs=1)
                nc.scalar.activation(utp[:, :], pU[:, :], AFT.Sigmoid,
                                     bias=cb_b[:, h:h + 1])
                nc.sync.dma_start(
                    io["out_unc"][h].rearrange("(mp p) n -> p mp n", p=128)[:, 2 * mp:2 * mp + 2, :],
                    utp[:, :].rearrange("p (a n) -> p a n", a=2))
                utps.append(utp)
                rtps.append(rtp)

            # combine + av
            pav = ps2.tile([64, R], F32, tag="av", name=f"pav{h}", bufs=2)
            for mp in range(TT // 2):
                atp = asb.tile([128, 512], F32R, tag=f"attnT{mp}",
                               name=f"atp{h}_{mp}", bufs=1)
                nc.vector.tensor_mul(utps[mp][:, :], utps[mp][:, :],
                                     rtps[mp][:, :])
                nc.vector.tensor_add(atp[:, :], amt[mp][:, :], utps[mp][:, :])
                for half in range(2):
                    m = 2 * mp + half
                    nc.tensor.matmul(pav[:, :],
                                     vtok[m][:, h * 64:(h + 1) * 64],
                                     atp[:, half * R:(half + 1) * R],
                                     start=(m == 0), stop=(m == TT - 1))
            nc.vector.tensor_copy(aoT[jj][po:po + 64, :], pav[:, :])

    es_kv.close()  # frees kT, v

    # ---------- phase 5: proj + residual + LN2 ----------
    mlp = ctx.enter_context(tc.tile_pool(name="mlp", bufs=1))
    ln2g_b = _bcast_load(nc, mlp, bc7[2], DIM, "ln2g")
    ln2b_b = _bcast_load(nc, mlp, bc7[3], DIM, "ln2b")
    projb_b = _bcast_load(nc, mlp, bc7[4], DIM, "projb")
    fc2b_b = _bcast_load(nc, mlp, bc7[5], DIM, "fc2b")
    x2 = [mlp.tile([128, DIM], F32, tag=f"x2_{t}", name=f"x2_{t}")
          for t in range(NT)]
    h2T = [mlp.tile([128, R], BF16, tag=f"h2T{j}", name=f"h2T{j}")
           for j in range(DT)]

    with tc.tile_pool(name="pj_sb", bufs=2) as pj_sb, \
         tc.tile_pool(name="ps_pj", bufs=2, space="PSUM") as ps_pj:
        wproj = []
        for dk in range(DT):
            wt = pj_sb.tile([128, DIM], BF16, tag=f"wproj{dk}", name=f"wproj{dk}",
                            bufs=1)
            nc.sync.dma_start(wt[:, :], io["w_proj"][dk * 128:(dk + 1) * 128, :])
            wproj.append(wt)
        for t in range(NT):
            nsl = slice(t * 128, (t + 1) * 128)
            pp = ps_pj.tile([128, DIM], F32, tag="ps_p", name=f"psp{t}")
            for fb, fw in ((0, 512), (512, 256)):
                for dk in range(DT):
                    nc.tensor.matmul(pp[:, fb:fb + fw],
                                     aoT[dk][:, nsl],
                                     wproj[dk][:, fb:fb + fw],
                                     start=(dk == 0), stop=(dk == DT - 1))
            # x2 = proj_out + b_proj + x
            nc.vector.scalar_tensor_tensor(x2[t][:, :], pp[:, :], 1.0,
                                           xres[t][:, :], op0=ALU.mult,
                                           op1=ALU.add)
            nc.vector.tensor_add(x2[t][:, :], x2[t][:, :], projb_b[:, :])
            # LN2 -> h2, transpose into h2T
            h2 = pj_sb.tile([128, DIM], F32, tag="h2", name=f"h2_{t}")
            _layernorm(nc, pj_sb, x2[t], ln2g_b, ln2b_b, h2, 10 + t, eps_col)
            for j in range(DT):
                pt = ps_pj.tile([128, 128], F32, tag="tp3", name=f"tph{t}_{j}")
                nc.tensor.transpose(pt[:, :], h2[:, j * 128:(j + 1) * 128],
                                    identity[:, :])
                nc.scalar.copy(h2T[j][:, nsl], pt[:, :])

    # ---------- phase 6: MLP ----------
    with tc.tile_pool(name="fc_sb", bufs=6) as fcs, \
         tc.tile_pool(name="ps_f1", bufs=4, space="PSUM") as ps_f1, \
         tc.tile_pool(name="ps_f2", bufs=2, space="PSUM") as ps_f2:
        # fc1 + gelu (feature-major): out [f128, 256]
        for ft in range(MT):
            pf = ps_f1.tile([128, R], F32, tag="ps_f1", name=f"psf1_{ft}")
            for dk in range(DT):
                nc.tensor.matmul(pf[:, :],
                                 wfc1[dk][:, ft * 128:(ft + 1) * 128],
                                 h2T[dk][:, :],
                                 start=(dk == 0), stop=(dk == DT - 1))
            nc.scalar.activation(g1[ft][:, :], pf[:, :], AFT.Gelu_apprx_tanh,
                                 bias=fc1b_plane[:, ft:ft + 1])
        # fc2 (token-major): out [n128, 768]
        for t in range(NT):
            nsl = slice(t * 128, (t + 1) * 128)
            pf2 = ps_f2.tile([128, DIM], F32, tag="ps_f2", name=f"psf2_{t}")
            for fb, fw in ((0, 512), (512, 256)):
                for mt in range(MT):
                    w2 = fcs.tile([128, fw], BF16, tag=f"wfc2_{fb}",
                                  name=f"wfc2_{t}_{fb}_{mt}")
                    nc.sync.dma_start(
                        w2[:, :],
                        io["w_fc2"][mt * 128:(mt + 1) * 128, fb:fb + fw])
                    nc.tensor.matmul(pf2[:, fb:fb + fw],
                                     g1[mt][:, nsl],
                                     w2[:, :],
                                     start=(mt == 0), stop=(mt == MT - 1))
            ot = fcs.tile([128, DIM], F32, tag="xout", name=f"xo{t}")
            nc.vector.scalar_tensor_tensor(ot[:, :], pf2[:, :], 1.0,
                                           x2[t][:, :], op0=ALU.mult,
                                           op1=ALU.add)
            nc.vector.tensor_add(ot[:, :], ot[:, :], fc2b_b[:, :])
            nc.sync.dma_start(io["out_x"][nsl, :], ot[:, :])


def kernel(**inputs):
    global LAST_RESULTS, LAST_PROGRAM
    x = np.ascontiguousarray(np.asarray(inputs["x"], dtype=np.float32))
    r = np.asarray(inputs["r"], dtype=np.float32)
    conv_w = np.asarray(inputs["conv_w"], dtype=np.float32)

    bf16 = ml_dtypes.bfloat16
    qkv_b = np.asarray(inputs["qkv_b"], np.float32)
    fc1_b = np.asarray(inputs["fc1_b"], np.float32)
    host = {
        "w_qkv": np.ascontiguousarray(
            np.asarray(inputs["qkv_w"], np.float32).T),
        "w_proj": np.ascontiguousarray(
            np.asarray(inputs["proj_w"], np.float32).T.astype(bf16)),
        "w_fc1": np.ascontiguousarray(
            np.asarray(inputs["fc1_w"], np.float32).T.astype(bf16)),
        "w_fc2": np.ascontiguousarray(
            np.asarray(inputs["fc2_w"], np.float32).T.astype(bf16)),
        "qkb_colt": np.ascontiguousarray(
            qkv_b[0:2 * DIM].reshape(2 * DIM // 128, 128).T),
        "fc1b_colt": np.ascontiguousarray(
            fc1_b.reshape(MLP // 128, 128).T),


        # cexp[h*HD+d, g] = conv_w[g, h]
        "cexp": np.ascontiguousarray(np.repeat(conv_w.T, HD, axis=0)),
    }
    bvecs = [inputs["ln1_g"], inputs["ln1_b"], inputs["ln2_g"], inputs["ln2_b"],
             inputs["proj_b"], inputs["fc2_b"],
             qkv_b[2 * DIM:3 * DIM]]
    host["bc7"] = np.ascontiguousarray(np.stack(
        [np.broadcast_to(np.asarray(v, np.float32), (128, DIM)) for v in bvecs]))
    host["cb_bc"] = np.ascontiguousarray(np.broadcast_to(
        np.asarray(inputs["conv_b"], np.float32), (128, HEADS)))
    host["onesrr"] = np.ones((2, 128), np.float32)

    perms = []
    in_maps = []
    for c in range(NCORES):
        b, rb = divmod(c, RB)
        n0 = rb * R
        perm = np.concatenate([np.arange(n0, n0 + R), np.arange(0, n0),
                               np.arange(n0 + R, N)])
        perms.append(perm)
        m = dict(host)
        m["xb"] = np.ascontiguousarray(x[b][perm])
        m["rs"] = np.ascontiguousarray(
            r[b][:, n0:n0 + R, :][:, :, perm].transpose(0, 2, 1))
        in_maps.append(m)

    global LAST_PROGRAM
    nc = _build_program()
    LAST_PROGRAM = (nc, in_maps)
    LAST_RESULTS = run_bass_kernel_spmd(nc, in_maps,
                                        core_ids=list(range(NCORES)))

    xo = np.empty((B, N, DIM), np.float32)
    am = np.empty((B, HEADS, N, N), np.float32)
    un = np.empty((B, HEADS, N, N), np.float32)
    for c in range(NCORES):
        b, rb = divmod(c, RB)
        n0 = rb * R
        res = LAST_RESULTS.results[c]
        xo[b, n0:n0 + R] = res["out_x"]
        am[b, :, n0:n0 + R][:, :, perms[c]] = res["out_am"].transpose(0, 2, 1)
        un[b, :, n0:n0 + R][:, :, perms[c]] = res["out_unc"].transpose(0, 2, 1)
    return xo, am, un


# revision 39
# speedup vs baseline: 1.0329x; 1.0329x over previous
"""Trainium2 Bass kernel for a dense transformer block (B=2, N=1024, D=768,
H=12, MLP=3072) returning (x_out, attn_mean, uncertainty).

Sharding: 8-way row-parallel. Core c handles batch b=c//4 and token rows
rb*256:(rb+1)*256 (rb=c%4). Each core redundantly computes LN1 + k/v over
the full sequence of its batch, so there are no collectives. Inputs are fed
with a per-core block-roll of the token axis so "our" rows are always rows
0:256 in the on-device view; the host un-permutes the key axis of the
attention outputs when gathering.

Head-mixing 1x1 conv trick: uncertainty_g = sigmoid(q~_g^T @ k + conv_b[g])
where q~_g[h*64+d, n] = conv_w[g, h] * qT[h*64+d, n] — a full K=768 matmul
instead of a cross-partition head reduction.
"""

import sys

if "/opt/trn_rl_repo" not in sys.path:
    sys.path.insert(0, "/opt/trn_rl_repo")

import ml_dtypes
import numpy as np
from contextlib import ExitStack

import concourse.bass as bass
import concourse.bacc as bacc
import concourse.mybir as mybir
from concourse import masks, tile
from concourse.bass_utils import run_bass_kernel_spmd

DIM, HEADS, MLP, B, N = 768, 12, 3072, 2, 1024
HD = DIM // HEADS            # 64
SCALE = float(HD) ** -0.5
NCORES = 8
RB = NCORES // B             # 4 row-blocks per batch
R = N // RB                  # 256 rows per core
NT = R // 128                # 2 n-tiles per core
TT = N // 128                # 8 token tiles (full sequence)
DT = DIM // 128              # 6 d-tiles
MT = MLP // 128              # 24 mlp tiles
EPS = 1e-5

F32 = mybir.dt.float32
BF16 = mybir.dt.bfloat16
F32R = mybir.dt.float32r
ALU = mybir.AluOpType
AFT = mybir.ActivationFunctionType
AX = mybir.AxisListType

LAST_RESULTS = None  # BassKernelResults stash for test harness introspection
LAST_PROGRAM = None  # (nc, in_maps) stash for test-harness benchmarking


def r32(ap):
    """Reinterpret an fp32 AP as float32r for full-rate PE matmuls."""
    return ap.bitcast(F32R)


def _build_program():
    nc = bacc.Bacc("TRN2", target_bir_lowering=False, debug=False,
                   num_devices=NCORES)

    # ---- DRAM I/O ----
    xb = nc.dram_tensor("xb", [N, DIM], F32, kind="ExternalInput").ap()
    rs = nc.dram_tensor("rs", [HEADS, N, R], F32, kind="ExternalInput").ap()
    w_qkv = nc.dram_tensor("w_qkv", [DIM, 3 * DIM], F32R, kind="ExternalInput").ap()
    w_proj = nc.dram_tensor("w_proj", [DIM, DIM], BF16, kind="ExternalInput").ap()
    w_fc1 = nc.dram_tensor("w_fc1", [DIM, MLP], BF16, kind="ExternalInput").ap()
    w_fc2 = nc.dram_tensor("w_fc2", [MLP, DIM], BF16, kind="ExternalInput").ap()
    qkb_colt = nc.dram_tensor("qkb_colt", [128, 2 * DIM // 128], F32,
                              kind="ExternalInput").ap()
    fc1b_colt = nc.dram_tensor("fc1b_colt", [128, MLP // 128], F32,
                               kind="ExternalInput").ap()
    cexp = nc.dram_tensor("cexp", [DIM, HEADS], F32, kind="ExternalInput").ap()
    bc7 = nc.dram_tensor("bc7", [7, 128, DIM], F32, kind="ExternalInput").ap()
    cb_bc = nc.dram_tensor("cb_bc", [128, HEADS], F32, kind="ExternalInput").ap()
    onesrr = nc.dram_tensor("onesrr", [2, 128], F32R, kind="ExternalInput").ap()

    out_x = nc.dram_tensor("out_x", [R, DIM], F32, kind="ExternalOutput").ap()
    out_am = nc.dram_tensor("out_am", [HEADS, N, R], F32, kind="ExternalOutput").ap()
    out_unc = nc.dram_tensor("out_unc", [HEADS, N, R], F32, kind="ExternalOutput").ap()

    with tile.TileContext(nc, num_cores=NCORES) as tc, ExitStack() as ctx:
        _emit(ctx, tc, nc, dict(
            xb=xb, rs=rs, w_qkv=w_qkv, w_proj=w_proj,
            w_fc1=w_fc1, w_fc2=w_fc2, qkb_colt=qkb_colt, fc1b_colt=fc1b_colt,
            cexp=cexp, bc7=bc7, cb_bc=cb_bc, onesrr=onesrr, out_x=out_x, out_am=out_am, out_unc=out_unc,
        ))
    nc.compile()
    return nc


def _bcast_load(nc, pool, dram_plane, n, tag):
    """DRAM [128, n] host-prebroadcast plane -> SBUF [128, n] tile."""
    out = pool.tile([128, n], F32, tag=tag, name=tag)
    nc.sync.dma_start(out[:, :], dram_plane)
    return out


def _col_plane(nc, pool, dram_plane, ncols, tag):
    """DRAM [128, ncols] host-preshaped bias-column plane -> SBUF tile.
    Column j holds bias[j*128:(j+1)*128]."""
    t = pool.tile([128, ncols], F32, tag=tag, name=tag)
    nc.sync.dma_start(t[:, :], dram_plane)
    return t


def _layernorm(nc, scratch, xt, g_b, b_b, out_t, idx, eps_col):
    """LN over free dim (768) of [128, 768] tile xt -> out_t."""
    mu = scratch.tile([128, 1], F32, tag="ln_mu", name=f"mu{idx}")
    nc.vector.reduce_sum(mu[:, :], xt[:, :], axis=AX.X)
    nc.vector.tensor_scalar_mul(mu[:, :], mu[:, :], 1.0 / DIM)
    # out = x - mu
    nc.vector.tensor_scalar_sub(out_t[:, :], xt[:, :], mu[:, :])
    sq = scratch.tile([128, DIM], F32, tag="ln_sq", name=f"sq{idx}")
    ssq = scratch.tile([128, 1], F32, tag="ln_ssq", name=f"ssq{idx}")
    nc.vector.scalar_tensor_tensor(sq[:, :], out_t[:, :], 1.0, out_t[:, :],
                                   op0=ALU.mult, op1=ALU.mult,
                                   accum_out=ssq[:, :])
    std = scratch.tile([128, 1], F32, tag="ln_std", name=f"std{idx}")
    # std = sqrt(ssq/DIM + eps)
    nc.scalar.activation(std[:, :], ssq[:, :], AFT.Sqrt, bias=eps_col[:, :],
                         scale=1.0 / DIM)
    rstd = scratch.tile([128, 1], F32, tag="ln_rstd", name=f"rstd{idx}")
    nc.vector.reciprocal(rstd[:, :], std[:, :])
    # out = (out * rstd) * g ; out += b
    nc.vector.scalar_tensor_tensor(out_t[:, :], out_t[:, :], rstd[:, :],
                                   g_b[:, :], op0=ALU.mult, op1=ALU.mult)
    nc.vector.tensor_add(out_t[:, :], out_t[:, :], b_b[:, :])


def _emit(ctx, tc, nc, io):
    xb, rs = io["xb"], io["rs"]

    # ---------- persistent pools ----------
    const = ctx.enter_context(tc.tile_pool(name="const", bufs=1))
    persist = ctx.enter_context(tc.tile_pool(name="persist", bufs=1))

    # x rows 0:256 (ours) first — LN1 is the head of the critical path
    xres = []
    for t in range(NT):
        xt = persist.tile([128, DIM], F32, tag=f"xres{t}", name=f"xres{t}")
        nc.sync.dma_start(xt[:, :], xb[t * 128:(t + 1) * 128, :])
        xres.append(xt)

    identity = const.tile([128, 128], F32, tag="ident", name="ident")
    masks.make_identity(nc, identity[:, :])
    eps_col = const.tile([128, 1], F32, tag="eps", name="eps")
    nc.gpsimd.memset(eps_col[:, :], EPS)
    bc7 = io["bc7"]
    cb_b = _bcast_load(nc, const, io["cb_bc"], HEADS, "cb")
    qkb_plane = _col_plane(nc, const, io["qkb_colt"], 2 * DIM // 128, "qkb")
    fc1b_plane = _col_plane(nc, const, io["fc1b_colt"], MLP // 128, "f1b")

    cexp_t = []
    for j in range(DT):
        t = const.tile([128, HEADS], F32, tag=f"cexp{j}", name=f"cexp{j}")
        nc.sync.dma_start(t[:, :], io["cexp"][j * 128:(j + 1) * 128, :])
        cexp_t.append(t)

    # qT [768, 256] and attn-out^T [768, 256], alive through attention
    qT = [persist.tile([128, R], F32R, tag=f"qT{j}", name=f"qT{j}")
          for j in range(DT)]
    aoT = [persist.tile([128, R], BF16, tag=f"aoT{j}", name=f"aoT{j}")
           for j in range(DT)]

    # ---------- phase 1+2: LN1 over full batch, h -> hT ----------
    # Pool stacking: left stack holds const/persist -> pool_h (closed after
    # qkv) -> attention scratch -> mlp. kT/v live on the right stack since
    # their lifetime (phases 3-4) straddles pool_h's close.
    es_h = ctx.enter_context(ExitStack())
    pool_h = es_h.enter_context(tc.tile_pool(name="pool_h", bufs=1))
    ln1g_b = _bcast_load(nc, pool_h, bc7[0], DIM, "ln1g")
    ln1b_b = _bcast_load(nc, pool_h, bc7[1], DIM, "ln1b")
    vb_b = _bcast_load(nc, pool_h, bc7[6], DIM, "vb")
    hT = [pool_h.tile([128, N], F32R, tag=f"hT{j}", name=f"hT{j}")
          for j in range(DT)]
    es_kv = ctx.enter_context(ExitStack())
    kv = es_kv.enter_context(tc.tile_pool(name="kv", bufs=1, side="right"))
    kT = [kv.tile([128, N], F32R, tag=f"kT{j}", name=f"kT{j}")
          for j in range(DT)]
    vtok = [kv.tile([128, DIM], F32R, tag=f"v{t}", name=f"v{t}")
            for t in range(TT)]

    with tc.tile_pool(name="ln1_scr", bufs=2) as scr, \
         tc.tile_pool(name="ps_t1", bufs=4, space="PSUM") as ps_t1:
        for t in range(TT):
            if t < NT:
                xt = xres[t]
            else:
                xt = scr.tile([128, DIM], F32, tag="x_full", name=f"x{t}")
                nc.sync.dma_start(xt[:, :], xb[t * 128:(t + 1) * 128, :])
            ht = scr.tile([128, DIM], F32, tag="h_full", name=f"h{t}")
            _layernorm(nc, scr, xt, ln1g_b, ln1b_b, ht, t, eps_col)
            for j in range(DT):
                pt = ps_t1.tile([128, 128], F32, tag="tp", name=f"tp{t}_{j}")
                nc.tensor.transpose(pt[:, :], ht[:, j * 128:(j + 1) * 128],
                                    identity[:, :])
                nc.scalar.copy(hT[j][:, t * 128:(t + 1) * 128], pt[:, :])

    # ---------- phase 3: qkv ----------
    with tc.tile_pool(name="wqkv", bufs=1, side="right") as wq_pool, \
         tc.tile_pool(name="ps_qkv", bufs=2, space="PSUM") as ps_qkv:
        wqkv = []
        for j in range(DT):
            wt = wq_pool.tile([128, 3 * DIM], F32R, tag=f"wqkv{j}",
                              name=f"wqkv{j}")
            nc.sync.dma_start(wt[:, :], io["w_qkv"][j * 128:(j + 1) * 128, :])
            wqkv.append(wt)

        # q (feature-major, our 256 rows): out [f128, 256]
        for j in range(DT):
            pq = ps_qkv.tile([128, R], F32, tag="ps_q", name=f"psq{j}",
                             bufs=1)
            for dk in range(DT):
                nc.tensor.matmul(pq[:, :],
                                 r32(wqkv[dk][:, j * 128:(j + 1) * 128]),
                                 r32(hT[dk][:, 0:R]),
                                 start=(dk == 0), stop=(dk == DT - 1))
            nc.scalar.add(qT[j][:, :], pq[:, :], qkb_plane[:, j:j + 1])

        # k (feature-major, full sequence): out [f128, 1024]
        for j in range(DT):
            pk = ps_qkv.tile([128, N], F32, tag="ps_k", name=f"psk{j}")
            for mb in range(2):
                sl = slice(mb * 512, (mb + 1) * 512)
                for dk in range(DT):
                    nc.tensor.matmul(
                        pk[:, sl],
                        r32(wqkv[dk][:, DIM + j * 128:DIM + (j + 1) * 128]),
                        r32(hT[dk][:, sl]),
                        start=(dk == 0), stop=(dk == DT - 1))
            nc.scalar.add(kT[j][:, :], pk[:, :],
                          qkb_plane[:, DT + j:DT + j + 1])

        # v (token-major, full sequence): out [tok128, 768]
        for t in range(TT):
            pv = ps_qkv.tile([128, DIM], F32, tag="ps_v", name=f"psv{t}",
                             bufs=1)
            for fb, fw in ((0, 512), (512, 256)):
                for dk in range(DT):
                    nc.tensor.matmul(
                        pv[:, fb:fb + fw],
                        r32(hT[dk][:, t * 128:(t + 1) * 128]),
                        r32(wqkv[dk][:, 2 * DIM + fb:2 * DIM + fb + fw]),
                        start=(dk == 0), stop=(dk == DT - 1))
            nc.vector.tensor_add(vtok[t][:, :], pv[:, :], vb_b[:, :])

    es_h.close()  # frees hT

    # ---------- MLP weight preload (hoistable into attention DMA idle) ----
    fcw = ctx.enter_context(tc.tile_pool(name="fc_w", bufs=1))
    wfc1 = []
    for dk in range(DT):
        wt = fcw.tile([128, MLP], BF16, tag=f"wfc1_{dk}", name=f"wfc1_{dk}")
        nc.sync.dma_start(wt[:, :], io["w_fc1"][dk * 128:(dk + 1) * 128, :])
        wfc1.append(wt)
    g1 = [fcw.tile([128, R], BF16, tag=f"g1_{ft}", name=f"g1_{ft}")
          for ft in range(MT)]

    # ---------- phase 4: attention ----------
    # Everything in [m, n] (transposed) layout. Per head: qk^T -> exp
    # (unnormalized) -> column sums via a ones-column matmul (cross-partition
    # reduce on PE) -> invB = 1/sums broadcast via rank-1 matmul ->
    # attn_mean^T = exp * invB; unc^T = sigmoid(k^T-contraction with the
    # conv-scaled q); attn^T = attn_mean^T + unc^T * r^T feeds av directly.
    # ACT runs only Exp and Sigmoid, grouped, so table reloads stay rare.
    ones_col = const.tile([1, 128], F32R, tag="ones_col", name="ones_col")
    nc.sync.dma_start(ones_col[:, :], io["onesrr"][0:1, :])
    ones128 = const.tile([128, 1], F32R, tag="ones128", name="ones128")
    nc.sync.dma_start(ones128[:, :],
                      io["onesrr"][1].rearrange("(p o) -> p o", o=1))
    with tc.tile_pool(name="at_sb", bufs=2) as asb, \
         tc.tile_pool(name="at_qg", bufs=2) as qg_pool, \
         tc.tile_pool(name="at_ps", bufs=2, space="PSUM") as ps2:
        for h in range(HEADS):
            jj, po = divmod(h, 2)
            po *= 64
            # q~_g = qT * cexp[:, h]  (for the uncertainty matmul)
            qg = [qg_pool.tile([128, R], F32R, tag=f"qg{j}", name=f"qg{h}_{j}")
                  for j in range(DT)]
            for j in range(DT):
                nc.vector.tensor_scalar_mul(qg[j][:, :],
                                            qT[j][:, :].bitcast(F32),
                                            cexp_t[j][:, h:h + 1])

            # exp(SCALE * qk^T), unnormalized, by m-pair
            exs = []
            for mp in range(TT // 2):
                pT = ps2.tile([128, 512], F32, tag="ps_qkt",
                              name=f"pqkt{h}_{mp}")
                for half in range(2):
                    m = 2 * mp + half
                    msl = slice(m * 128, (m + 1) * 128)
                    nc.tensor.matmul(pT[:, half * R:(half + 1) * R],
                                     kT[jj][po:po + 64, msl],
                                     qT[jj][po:po + 64, 0:R],
                                     start=True, stop=True)
                ex = asb.tile([128, 512], F32R, tag=f"ex{mp}",
                              name=f"ex{h}_{mp}", bufs=2)
                nc.scalar.activation(ex[:, :], pT[:, :], AFT.Exp, scale=SCALE)
                exs.append(ex)
            # column sums -> inv -> invB broadcast
            pss = ps2.tile([128, 512], F32, tag="ps_small", name=f"pss{h}",
                           bufs=1)
            for mp in range(TT // 2):
                for half in range(2):
                    nc.tensor.matmul(pss[0:1, 0:R], ones128[:, :],
                                     exs[mp][:, half * R:(half + 1) * R],
                                     start=(mp == 0 and half == 0),
                                     stop=(mp == TT // 2 - 1 and half == 1))
            invrow = asb.tile([1, R], F32R, tag="invrow", name=f"ivr{h}")
            with nc.allow_low_precision(reason="f32r rounding for PE rank-1"):
                nc.vector.reciprocal(invrow[:, :], pss[0:1, 0:R])
            psb = ps2.tile([128, 512], F32, tag="ps_small", name=f"psb{h}",
                           bufs=1)
            for half in range(2):
                nc.tensor.matmul(psb[:, half * R:(half + 1) * R],
                                 ones_col[:, :], invrow[:, :],
                                 start=True, stop=True)
            invB = asb.tile([128, 512], F32, tag="invB", name=f"invB{h}")
            nc.vector.tensor_copy(invB[:, :], psb[:, :])

            # attn_mean^T = exp * invB
            amt = []
            for mp in range(TT // 2):
                a = asb.tile([128, 512], F32, tag=f"amt{mp}",
                             name=f"amt{h}_{mp}", bufs=2)
                nc.vector.tensor_mul(a[:, :], exs[mp][:, :].bitcast(F32),
                                     invB[:, :])
                nc.sync.dma_start(
                    io["out_am"][h].rearrange("(mp p) n -> p mp n", p=128)[:, 2 * mp:2 * mp + 2, :],
                    a[:, :].rearrange("p (a n) -> p a n", a=2))
                amt.append(a)

            # unc^T
            utps = []
            rtps = []
            for mp in range(TT // 2):
                rtp = asb.tile([128, 512], F32, tag=f"rt{mp}",
                               name=f"rt{h}_{mp}", bufs=3)
                nc.sync.dma_start(
                    rtp[:, :].rearrange("p (a n) -> p a n", a=2),
                    rs[h].rearrange("(mp p) n -> p mp n", p=128)[:, 2 * mp:2 * mp + 2, :])
                pU = ps2.tile([128, 512], F32, tag="ps_unc",
                              name=f"pun{h}_{mp}")
                for half in range(2):
                    m = 2 * mp + half
                    msl = slice(m * 128, (m + 1) * 128)
                    for dk in range(DT):
                        nc.tensor.matmul(pU[:, half * R:(half + 1) * R],
                                         kT[dk][:, msl],
                                         qg[dk][:, 0:R],
                                         start=(dk == 0), stop=(dk == DT - 1))
                utp = asb.tile([128, 512], F32, tag=f"unc{mp}",
                               name=f"ut{h}_{mp}", bufs=1)
                nc.scalar.activation(utp[:, :], pU[:, :], AFT.Sigmoid,
                                     bias=cb_b[:, h:h + 1])
                nc.sync.dma_start(
                    io["out_unc"][h].rearrange("(mp p) n -> p mp n", p=128)[:, 2 * mp:2 * mp + 2, :],
                    utp[:, :].rearrange("p (a n) -> p a n", a=2))
                utps.append(utp)
                rtps.append(rtp)

            # combine + av
            pav = ps2.tile([64, R], F32, tag="av", name=f"pav{h}", bufs=2)
            for mp in range(TT // 2):
                atp = asb.tile([128, 512], F32R, tag=f"attnT{mp}",
                               name=f"atp{h}_{mp}", bufs=1)
                nc.vector.tensor_mul(utps[mp][:, :], utps[mp][:, :],
                                     rtps[mp][:, :])
                nc.vector.tensor_add(atp[:, :], amt[mp][:, :], utps[mp][:, :])
                for half in range(2):
                    m = 2 * mp + half
                    nc.tensor.matmul(pav[:, :],
                                     vtok[m][:, h * 64:(h + 1) * 64],
                                     atp[:, half * R:(half + 1) * R],
                                     start=(m == 0), stop=(m == TT - 1))
            nc.vector.tensor_copy(aoT[jj][po:po + 64, :], pav[:, :])

    es_kv.close()  # frees kT, v

    # ---------- phase 5: proj + residual + LN2 ----------
    mlp = ctx.enter_context(tc.tile_pool(name="mlp", bufs=1))
    ln2g_b = _bcast_load(nc, mlp, bc7[2], DIM, "ln2g")
    ln2b_b = _bcast_load(nc, mlp, bc7[3], DIM, "ln2b")
    projb_b = _bcast_load(nc, mlp, bc7[4], DIM, "projb")
    fc2b_b = _bcast_load(nc, mlp, bc7[5], DIM, "fc2b")
    x2 = [mlp.tile([128, DIM], F32, tag=f"x2_{t}", name=f"x2_{t}")
          for t in range(NT)]
    h2T = [mlp.tile([128, R], BF16, tag=f"h2T{j}", name=f"h2T{j}")
           for j in range(DT)]

    with tc.tile_pool(name="pj_sb", bufs=2) as pj_sb, \
         tc.tile_pool(name="ps_pj", bufs=2, space="PSUM") as ps_pj:
        wproj = []
        for dk in range(DT):
            wt = pj_sb.tile([128, DIM], BF16, tag=f"wproj{dk}", name=f"wproj{dk}",
                            bufs=1)
            nc.sync.dma_start(wt[:, :], io["w_proj"][dk * 128:(dk + 1) * 128, :])
            wproj.append(wt)
        for t in range(NT):
            nsl = slice(t * 128, (t + 1) * 128)
            pp = ps_pj.tile([128, DIM], F32, tag="ps_p", name=f"psp{t}")
            for fb, fw in ((0, 512), (512, 256)):
                for dk in range(DT):
                    nc.tensor.matmul(pp[:, fb:fb + fw],
                                     aoT[dk][:, nsl],
                                     wproj[dk][:, fb:fb + fw],
                                     start=(dk == 0), stop=(dk == DT - 1))
            # x2 = proj_out + b_proj + x
            nc.vector.scalar_tensor_tensor(x2[t][:, :], pp[:, :], 1.0,
                                           xres[t][:, :], op0=ALU.mult,
                                           op1=ALU.add)
            nc.vector.tensor_add(x2[t][:, :], x2[t][:, :], projb_b[:, :])
            # LN2 -> h2, transpose into h2T
            h2 = pj_sb.tile([128, DIM], F32, tag="h2", name=f"h2_{t}")
            _layernorm(nc, pj_sb, x2[t], ln2g_b, ln2b_b, h2, 10 + t, eps_col)
            for j in range(DT):
                pt = ps_pj.tile([128, 128], F32, tag="tp3", name=f"tph{t}_{j}")
                nc.tensor.transpose(pt[:, :], h2[:, j * 128:(j + 1) * 128],
                                    identity[:, :])
                nc.scalar.copy(h2T[j][:, nsl], pt[:, :])

    # ---------- phase 6: MLP ----------
    with tc.tile_pool(name="fc_sb", bufs=6) as fcs, \
         tc.tile_pool(name="ps_f1", bufs=4, space="PSUM") as ps_f1, \
         tc.tile_pool(name="ps_f2", bufs=2, space="PSUM") as ps_f2:
        # fc1 + gelu (feature-major): out [f128, 256]
        for ft in range(MT):
            pf = ps_f1.tile([128, R], F32, tag="ps_f1", name=f"psf1_{ft}")
            for dk in range(DT):
                nc.tensor.matmul(pf[:, :],
                                 wfc1[dk][:, ft * 128:(ft + 1) * 128],
                                 h2T[dk][:, :],
                                 start=(dk == 0), stop=(dk == DT - 1))
            nc.scalar.activation(g1[ft][:, :], pf[:, :], AFT.Gelu_apprx_tanh,
                                 bias=fc1b_plane[:, ft:ft + 1])
        # fc2 (token-major): out [n128, 768]
        for t in range(NT):
            nsl = slice(t * 128, (t + 1) * 128)
            pf2 = ps_f2.tile([128, DIM], F32, tag="ps_f2", name=f"psf2_{t}")
            for fb, fw in ((0, 512), (512, 256)):
                for mt in range(MT):
                    w2 = fcs.tile([128, fw], BF16, tag=f"wfc2_{fb}",
                                  name=f"wfc2_{t}_{fb}_{mt}")
                    nc.sync.dma_start(
                        w2[:, :],
                        io["w_fc2"][mt * 128:(mt + 1) * 128, fb:fb + fw])
                    nc.tensor.matmul(pf2[:, fb:fb + fw],
                                     g1[mt][:, nsl],
                                     w2[:, :],
                                     start=(mt == 0), stop=(mt == MT - 1))
            ot = fcs.tile([128, DIM], F32, tag="xout", name=f"xo{t}")
            nc.vector.scalar_tensor_tensor(ot[:, :], pf2[:, :], 1.0,
                                           x2[t][:, :], op0=ALU.mult,
                                           op1=ALU.add)
            nc.vector.tensor_add(ot[:, :], ot[:, :], fc2b_b[:, :])
            nc.sync.dma_start(io["out_x"][nsl, :], ot[:, :])


def kernel(**inputs):
    global LAST_RESULTS, LAST_PROGRAM
    x = np.ascontiguousarray(np.asarray(inputs["x"], dtype=np.float32))
    r = np.asarray(inputs["r"], dtype=np.float32)
    conv_w = np.asarray(inputs["conv_w"], dtype=np.float32)

    bf16 = ml_dtypes.bfloat16
    qkv_b = np.asarray(inputs["qkv_b"], np.float32)
    fc1_b = np.asarray(inputs["fc1_b"], np.float32)
    host = {
        "w_qkv": np.ascontiguousarray(
            np.asarray(inputs["qkv_w"], np.float32).T),
        "w_proj": np.ascontiguousarray(
            np.asarray(inputs["proj_w"], np.float32).T.astype(bf16)),
        "w_fc1": np.ascontiguousarray(
            np.asarray(inputs["fc1_w"], np.float32).T.astype(bf16)),
        "w_fc2": np.ascontiguousarray(
            np.asarray(inputs["fc2_w"], np.float32).T.astype(bf16)),
        "qkb_colt": np.ascontiguousarray(
            qkv_b[0:2 * DIM].reshape(2 * DIM // 128, 128).T),
        "fc1b_colt": np.ascontiguousarray(
            fc1_b.reshape(MLP // 128, 128).T),


        # cexp[h*HD+d, g] = conv_w[g, h]
        "cexp": np.ascontiguousarray(np.repeat(conv_w.T, HD, axis=0)),
    }
    bvecs = [inputs["ln1_g"], inputs["ln1_b"], inputs["ln2_g"], inputs["ln2_b"],
             inputs["proj_b"], inputs["fc2_b"],
             qkv_b[2 * DIM:3 * DIM]]
    host["bc7"] = np.ascontiguousarray(np.stack(
        [np.broadcast_to(np.asarray(v, np.float32), (128, DIM)) for v in bvecs]))
    host["cb_bc"] = np.ascontiguousarray(np.broadcast_to(
        np.asarray(inputs["conv_b"], np.float32), (128, HEADS)))
    host["onesrr"] = np.ones((2, 128), np.float32)

    perms = []
    in_maps = []
    for c in range(NCORES):
        b, rb = divmod(c, RB)
        n0 = rb * R
        perm = np.concatenate([np.arange(n0, n0 + R), np.arange(0, n0),
                               np.arange(n0 + R, N)])
        perms.append(perm)
        m = dict(host)
        m["xb"] = np.ascontiguousarray(x[b][perm])
        m["rs"] = np.ascontiguousarray(
            r[b][:, n0:n0 + R, :][:, :, perm].transpose(0, 2, 1))
        in_maps.append(m)

    global LAST_PROGRAM
    nc = _build_program()
    LAST_PROGRAM = (nc, in_maps)
    LAST_RESULTS = run_bass_kernel_spmd(nc, in_maps,
                                        core_ids=list(range(NCORES)))

    xo = np.empty((B, N, DIM), np.float32)
    am = np.empty((B, HEADS, N, N), np.float32)
    un = np.empty((B, HEADS, N, N), np.float32)
    for c in range(NCORES):
        b, rb = divmod(c, RB)
        n0 = rb * R
        res = LAST_RESULTS.results[c]
        xo[b, n0:n0 + R] = res["out_x"]
        am[b, :, n0:n0 + R][:, :, perms[c]] = res["out_am"].transpose(0, 2, 1)
        un[b, :, n0:n0 + R][:, :, perms[c]] = res["out_unc"].transpose(0, 2, 1)
    return xo, am, un


# revision 43
# speedup vs baseline: 1.0464x; 1.0130x over previous
"""Trainium2 Bass kernel for a dense transformer block (B=2, N=1024, D=768,
H=12, MLP=3072) returning (x_out, attn_mean, uncertainty).

Sharding: 8-way row-parallel. Core c handles batch b=c//4 and token rows
rb*256:(rb+1)*256 (rb=c%4). Each core redundantly computes LN1 + k/v over
the full sequence of its batch, so there are no collectives. Inputs are fed
with a per-core block-roll of the token axis so "our" rows are always rows
0:256 in the on-device view; the host un-permutes the key axis of the
attention outputs when gathering.

Head-mixing 1x1 conv trick: uncertainty_g = sigmoid(q~_g^T @ k + conv_b[g])
where q~_g[h*64+d, n] = conv_w[g, h] * qT[h*64+d, n] — a full K=768 matmul
instead of a cross-partition head reduction.
"""

import sys

if "/opt/trn_rl_repo" not in sys.path:
    sys.path.insert(0, "/opt/trn_rl_repo")

import ml_dtypes
import numpy as np
from contextlib import ExitStack

import concourse.bass as bass
import concourse.bacc as bacc
import concourse.mybir as mybir
from concourse import masks, tile
from concourse.bass_utils import run_bass_kernel_spmd

DIM, HEADS, MLP, B, N = 768, 12, 3072, 2, 1024
HD = DIM // HEADS            # 64
SCALE = float(HD) ** -0.5
NCORES = 8
RB = NCORES // B             # 4 row-blocks per batch
R = N // RB                  # 256 rows per core
NT = R // 128                # 2 n-tiles per core
TT = N // 128                # 8 token tiles (full sequence)
DT = DIM // 128              # 6 d-tiles
MT = MLP // 128              # 24 mlp tiles
EPS = 1e-5

F32 = mybir.dt.float32
BF16 = mybir.dt.bfloat16
F32R = mybir.dt.float32r
ALU = mybir.AluOpType
AFT = mybir.ActivationFunctionType
AX = mybir.AxisListType

LAST_RESULTS = None  # BassKernelResults stash for test harness introspection
LAST_PROGRAM = None  # (nc, in_maps) stash for test-harness benchmarking


def r32(ap):
    """Reinterpret an fp32 AP as float32r for full-rate PE matmuls."""
    return ap.bitcast(F32R)


def _build_program():
    nc = bacc.Bacc("TRN2", target_bir_lowering=False, debug=False,
                   num_devices=NCORES)

    # ---- DRAM I/O ----
    xb = nc.dram_tensor("xb", [N, DIM], F32, kind="ExternalInput").ap()
    rs = nc.dram_tensor("rs", [HEADS, N, R], F32, kind="ExternalInput").ap()
    w_qkv = nc.dram_tensor("w_qkv", [DIM, 3 * DIM], F32R, kind="ExternalInput").ap()
    w_proj = nc.dram_tensor("w_proj", [DIM, DIM], BF16, kind="ExternalInput").ap()
    w_fc1 = nc.dram_tensor("w_fc1", [DIM, MLP], BF16, kind="ExternalInput").ap()
    w_fc2 = nc.dram_tensor("w_fc2", [MLP, DIM], BF16, kind="ExternalInput").ap()
    qkb_colt = nc.dram_tensor("qkb_colt", [128, 2 * DIM // 128], F32,
                              kind="ExternalInput").ap()
    fc1b_colt = nc.dram_tensor("fc1b_colt", [128, MLP // 128], F32,
                               kind="ExternalInput").ap()
    cexp = nc.dram_tensor("cexp", [DIM, HEADS], F32, kind="ExternalInput").ap()
    bc7 = nc.dram_tensor("bc7", [7, 128, DIM], F32, kind="ExternalInput").ap()
    cb_bc = nc.dram_tensor("cb_bc", [128, HEADS], F32, kind="ExternalInput").ap()
    onesrr = nc.dram_tensor("onesrr", [2, 128], F32R, kind="ExternalInput").ap()

    out_x = nc.dram_tensor("out_x", [R, DIM], F32, kind="ExternalOutput").ap()
    out_am = nc.dram_tensor("out_am", [HEADS, N, R], F32, kind="ExternalOutput").ap()
    out_unc = nc.dram_tensor("out_unc", [HEADS, N, R], F32, kind="ExternalOutput").ap()

    with tile.TileContext(nc, num_cores=NCORES) as tc, ExitStack() as ctx:
        _emit(ctx, tc, nc, dict(
            xb=xb, rs=rs, w_qkv=w_qkv, w_proj=w_proj,
            w_fc1=w_fc1, w_fc2=w_fc2, qkb_colt=qkb_colt, fc1b_colt=fc1b_colt,
            cexp=cexp, bc7=bc7, cb_bc=cb_bc, onesrr=onesrr, out_x=out_x, out_am=out_am, out_unc=out_unc,
        ))
    nc.compile()
    return nc


def _bcast_load(nc, pool, dram_plane, n, tag):
    """DRAM [128, n] host-prebroadcast plane -> SBUF [128, n] tile."""
    out = pool.tile([128, n], F32, tag=tag, name=tag)
    nc.sync.dma_start(out[:, :], dram_plane)
    return out


def _col_plane(nc, pool, dram_plane, ncols, tag):
    """DRAM [128, ncols] host-preshaped bias-column plane -> SBUF tile.
    Column j holds bias[j*128:(j+1)*128]."""
    t = pool.tile([128, ncols], F32, tag=tag, name=tag)
    nc.sync.dma_start(t[:, :], dram_plane)
    return t


def _layernorm(nc, scratch, xt, g_b, b_b, out_t, idx, eps_col):
    """LN over free dim (768) of [128, 768] tile xt -> out_t."""
    mu = scratch.tile([128, 1], F32, tag="ln_mu", name=f"mu{idx}")
    nc.vector.reduce_sum(mu[:, :], xt[:, :], axis=AX.X)
    nc.vector.tensor_scalar_mul(mu[:, :], mu[:, :], 1.0 / DIM)
    # out = x - mu
    nc.vector.tensor_scalar_sub(out_t[:, :], xt[:, :], mu[:, :])
    sq = scratch.tile([128, DIM], F32, tag="ln_sq", name=f"sq{idx}")
    ssq = scratch.tile([128, 1], F32, tag="ln_ssq", name=f"ssq{idx}")
    nc.vector.scalar_tensor_tensor(sq[:, :], out_t[:, :], 1.0, out_t[:, :],
                                   op0=ALU.mult, op1=ALU.mult,
                                   accum_out=ssq[:, :])
    std = scratch.tile([128, 1], F32, tag="ln_std", name=f"std{idx}")
    # std = sqrt(ssq/DIM + eps)
    nc.scalar.activation(std[:, :], ssq[:, :], AFT.Sqrt, bias=eps_col[:, :],
                         scale=1.0 / DIM)
    rstd = scratch.tile([128, 1], F32, tag="ln_rstd", name=f"rstd{idx}")
    nc.vector.reciprocal(rstd[:, :], std[:, :])
    # out = (out * rstd) * g ; out += b
    nc.vector.scalar_tensor_tensor(out_t[:, :], out_t[:, :], rstd[:, :],
                                   g_b[:, :], op0=ALU.mult, op1=ALU.mult)
    nc.vector.tensor_add(out_t[:, :], out_t[:, :], b_b[:, :])


def _emit(ctx, tc, nc, io):
    xb, rs = io["xb"], io["rs"]

    # ---------- persistent pools ----------
    const = ctx.enter_context(tc.tile_pool(name="const", bufs=1))
    persist = ctx.enter_context(tc.tile_pool(name="persist", bufs=1))

    # x rows 0:256 (ours) first — LN1 is the head of the critical path
    xres = []
    for t in range(NT):
        xt = persist.tile([128, DIM], F32, tag=f"xres{t}", name=f"xres{t}")
        nc.sync.dma_start(xt[:, :], xb[t * 128:(t + 1) * 128, :])
        xres.append(xt)

    identity = const.tile([128, 128], F32, tag="ident", name="ident")
    masks.make_identity(nc, identity[:, :])
    eps_col = const.tile([128, 1], F32, tag="eps", name="eps")
    nc.gpsimd.memset(eps_col[:, :], EPS)
    bc7 = io["bc7"]
    cb_b = _bcast_load(nc, const, io["cb_bc"], HEADS, "cb")
    qkb_plane = _col_plane(nc, const, io["qkb_colt"], 2 * DIM // 128, "qkb")
    fc1b_plane = _col_plane(nc, const, io["fc1b_colt"], MLP // 128, "f1b")

    cexp_t = []
    for j in range(DT):
        t = const.tile([128, HEADS], F32, tag=f"cexp{j}", name=f"cexp{j}")
        nc.sync.dma_start(t[:, :], io["cexp"][j * 128:(j + 1) * 128, :])
        cexp_t.append(t)

    # qT [768, 256] and attn-out^T [768, 256], alive through attention
    qT = [persist.tile([128, R], F32R, tag=f"qT{j}", name=f"qT{j}")
          for j in range(DT)]
    aoT = [persist.tile([128, R], BF16, tag=f"aoT{j}", name=f"aoT{j}")
           for j in range(DT)]

    # ---------- phase 1+2: LN1 over full batch, h -> hT ----------
    # Pool stacking: left stack holds const/persist -> pool_h (closed after
    # qkv) -> attention scratch -> mlp. kT/v live on the right stack since
    # their lifetime (phases 3-4) straddles pool_h's close.
    es_h = ctx.enter_context(ExitStack())
    pool_h = es_h.enter_context(tc.tile_pool(name="pool_h", bufs=1))
    ln1g_b = _bcast_load(nc, pool_h, bc7[0], DIM, "ln1g")
    ln1b_b = _bcast_load(nc, pool_h, bc7[1], DIM, "ln1b")
    vb_b = _bcast_load(nc, pool_h, bc7[6], DIM, "vb")
    hT = [pool_h.tile([128, N], F32R, tag=f"hT{j}", name=f"hT{j}")
          for j in range(DT)]
    es_kv = ctx.enter_context(ExitStack())
    kv = es_kv.enter_context(tc.tile_pool(name="kv", bufs=1, side="right"))
    kT = [kv.tile([128, N], F32R, tag=f"kT{j}", name=f"kT{j}")
          for j in range(DT)]
    vtok = [kv.tile([128, DIM], F32R, tag=f"v{t}", name=f"v{t}")
            for t in range(TT)]

    with tc.tile_pool(name="ln1_scr", bufs=2) as scr, \
         tc.tile_pool(name="ps_t1", bufs=4, space="PSUM") as ps_t1:
        for t in range(TT):
            if t < NT:
                xt = xres[t]
            else:
                xt = scr.tile([128, DIM], F32, tag="x_full", name=f"x{t}")
                nc.sync.dma_start(xt[:, :], xb[t * 128:(t + 1) * 128, :])
            ht = scr.tile([128, DIM], F32, tag="h_full", name=f"h{t}")
            _layernorm(nc, scr, xt, ln1g_b, ln1b_b, ht, t, eps_col)
            for j in range(DT):
                pt = ps_t1.tile([128, 128], F32, tag="tp", name=f"tp{t}_{j}")
                nc.tensor.transpose(pt[:, :], ht[:, j * 128:(j + 1) * 128],
                                    identity[:, :])
                nc.scalar.copy(hT[j][:, t * 128:(t + 1) * 128], pt[:, :])

    # ---------- phase 3: qkv ----------
    with tc.tile_pool(name="wqkv", bufs=1, side="right") as wq_pool, \
         tc.tile_pool(name="ps_qkv", bufs=2, space="PSUM") as ps_qkv:
        wqkv = []
        for j in range(DT):
            wt = wq_pool.tile([128, 3 * DIM], F32R, tag=f"wqkv{j}",
                              name=f"wqkv{j}")
            nc.sync.dma_start(wt[:, :], io["w_qkv"][j * 128:(j + 1) * 128, :])
            wqkv.append(wt)

        # q (feature-major, our 256 rows): out [f128, 256]
        for j in range(DT):
            pq = ps_qkv.tile([128, R], F32, tag="ps_q", name=f"psq{j}",
                             bufs=1)
            for dk in range(DT):
                nc.tensor.matmul(pq[:, :],
                                 r32(wqkv[dk][:, j * 128:(j + 1) * 128]),
                                 r32(hT[dk][:, 0:R]),
                                 start=(dk == 0), stop=(dk == DT - 1))
            nc.scalar.add(qT[j][:, :], pq[:, :], qkb_plane[:, j:j + 1])

        # k (feature-major, full sequence): out [f128, 1024]
        for j in range(DT):
            pk = ps_qkv.tile([128, N], F32, tag="ps_k", name=f"psk{j}")
            for mb in range(2):
                sl = slice(mb * 512, (mb + 1) * 512)
                for dk in range(DT):
                    nc.tensor.matmul(
                        pk[:, sl],
                        r32(wqkv[dk][:, DIM + j * 128:DIM + (j + 1) * 128]),
                        r32(hT[dk][:, sl]),
                        start=(dk == 0), stop=(dk == DT - 1))
            nc.scalar.add(kT[j][:, :], pk[:, :],
                          qkb_plane[:, DT + j:DT + j + 1])

        # v (token-major, full sequence): out [tok128, 768]
        for t in range(TT):
            pv = ps_qkv.tile([128, DIM], F32, tag="ps_v", name=f"psv{t}",
                             bufs=1)
            for fb, fw in ((0, 512), (512, 256)):
                for dk in range(DT):
                    nc.tensor.matmul(
                        pv[:, fb:fb + fw],
                        r32(hT[dk][:, t * 128:(t + 1) * 128]),
                        r32(wqkv[dk][:, 2 * DIM + fb:2 * DIM + fb + fw]),
                        start=(dk == 0), stop=(dk == DT - 1))
            nc.vector.tensor_add(vtok[t][:, :], pv[:, :], vb_b[:, :])

    es_h.close()  # frees hT

    # ---------- MLP weight preload (hoistable into attention DMA idle) ----
    fcw = ctx.enter_context(tc.tile_pool(name="fc_w", bufs=1))
    wfc1 = []
    for dk in range(DT):
        wt = fcw.tile([128, MLP], BF16, tag=f"wfc1_{dk}", name=f"wfc1_{dk}")
        nc.sync.dma_start(wt[:, :], io["w_fc1"][dk * 128:(dk + 1) * 128, :])
        wfc1.append(wt)

    # ---------- phase 4: attention ----------
    # Everything in [m, n] (transposed) layout. Per head: qk^T -> exp
    # (unnormalized) -> column sums via a ones-column matmul (cross-partition
    # reduce on PE) -> invB = 1/sums broadcast via rank-1 matmul ->
    # attn_mean^T = exp * invB; unc^T = sigmoid(k^T-contraction with the
    # conv-scaled q); attn^T = attn_mean^T + unc^T * r^T feeds av directly.
    # ACT runs only Exp and Sigmoid, grouped, so table reloads stay rare.
    ones_col = const.tile([1, 128], F32R, tag="ones_col", name="ones_col")
    nc.sync.dma_start(ones_col[:, :], io["onesrr"][0:1, :])
    ones128 = const.tile([128, 1], F32R, tag="ones128", name="ones128")
    nc.sync.dma_start(ones128[:, :],
                      io["onesrr"][1].rearrange("(p o) -> p o", o=1))
    with tc.tile_pool(name="at_sb", bufs=2) as asb, \
         tc.tile_pool(name="at_qg", bufs=2) as qg_pool, \
         tc.tile_pool(name="at_ps", bufs=2, space="PSUM") as ps2:
        for h in range(HEADS):
            jj, po = divmod(h, 2)
            po *= 64
            # q~_g = qT * cexp[:, h]  (for the uncertainty matmul)
            qg = [qg_pool.tile([128, R], F32R, tag=f"qg{j}", name=f"qg{h}_{j}")
                  for j in range(DT)]
            for j in range(DT):
                nc.vector.tensor_scalar_mul(qg[j][:, :],
                                            qT[j][:, :].bitcast(F32),
                                            cexp_t[j][:, h:h + 1])

            # exp(SCALE * qk^T), unnormalized, by m-pair
            exs = []
            for mp in range(TT // 2):
                pT = ps2.tile([128, 512], F32, tag="ps_qkt",
                              name=f"pqkt{h}_{mp}")
                for half in range(2):
                    m = 2 * mp + half
                    msl = slice(m * 128, (m + 1) * 128)
                    nc.tensor.matmul(pT[:, half * R:(half + 1) * R],
                                     kT[jj][po:po + 64, msl],
                                     qT[jj][po:po + 64, 0:R],
                                     start=True, stop=True)
                ex = asb.tile([128, 512], F32R, tag=f"ex{mp}",
                              name=f"ex{h}_{mp}", bufs=2)
                nc.scalar.activation(ex[:, :], pT[:, :], AFT.Exp, scale=SCALE)
                exs.append(ex)
            # column sums -> inv -> invB broadcast
            pss = ps2.tile([128, 512], F32, tag="ps_small", name=f"pss{h}",
                           bufs=1)
            for mp in range(TT // 2):
                for half in range(2):
                    nc.tensor.matmul(pss[0:1, 0:R], ones128[:, :],
                                     exs[mp][:, half * R:(half + 1) * R],
                                     start=(mp == 0 and half == 0),
                                     stop=(mp == TT // 2 - 1 and half == 1))
            invrow = asb.tile([1, R], F32R, tag="invrow", name=f"ivr{h}")
            with nc.allow_low_precision(reason="f32r rounding for PE rank-1"):
                nc.vector.reciprocal(invrow[:, :], pss[0:1, 0:R])
            psb = ps2.tile([128, 512], F32, tag="ps_small", name=f"psb{h}",
                           bufs=1)
            for half in range(2):
                nc.tensor.matmul(psb[:, half * R:(half + 1) * R],
                                 ones_col[:, :], invrow[:, :],
                                 start=True, stop=True)
            invB = asb.tile([128, 512], F32, tag="invB", name=f"invB{h}")
            nc.vector.tensor_copy(invB[:, :], psb[:, :])

            # attn_mean^T = exp * invB
            amt = []
            for mp in range(TT // 2):
                a = asb.tile([128, 512], F32, tag=f"amt{mp}",
                             name=f"amt{h}_{mp}", bufs=2)
                nc.vector.tensor_mul(a[:, :], exs[mp][:, :].bitcast(F32),
                                     invB[:, :])
                nc.sync.dma_start(
                    io["out_am"][h].rearrange("(mp p) n -> p mp n", p=128)[:, 2 * mp:2 * mp + 2, :],
                    a[:, :].rearrange("p (a n) -> p a n", a=2))
                amt.append(a)

            # unc^T
            utps = []
            rtps = []
            for mp in range(TT // 2):
                rtp = asb.tile([128, 512], F32, tag=f"rt{mp}",
                               name=f"rt{h}_{mp}", bufs=3)
                nc.sync.dma_start(
                    rtp[:, :].rearrange("p (a n) -> p a n", a=2),
                    rs[h].rearrange("(mp p) n -> p mp n", p=128)[:, 2 * mp:2 * mp + 2, :])
                pU = ps2.tile([128, 512], F32, tag="ps_unc",
                              name=f"pun{h}_{mp}", bufs=3)
                for half in range(2):
                    m = 2 * mp + half
                    msl = slice(m * 128, (m + 1) * 128)
                    for dk in range(DT):
                        nc.tensor.matmul(pU[:, half * R:(half + 1) * R],
                                         kT[dk][:, msl],
                                         qg[dk][:, 0:R],
                                         start=(dk == 0), stop=(dk == DT - 1))
                utp = asb.tile([128, 512], F32, tag=f"unc{mp}",
                               name=f"ut{h}_{mp}", bufs=2)
                nc.scalar.activation(utp[:, :], pU[:, :], AFT.Sigmoid,
                                     bias=cb_b[:, h:h + 1])
                nc.sync.dma_start(
                    io["out_unc"][h].rearrange("(mp p) n -> p mp n", p=128)[:, 2 * mp:2 * mp + 2, :],
                    utp[:, :].rearrange("p (a n) -> p a n", a=2))
                utps.append(utp)
                rtps.append(rtp)

            # combine + av
            pav = ps2.tile([64, R], F32, tag="av", name=f"pav{h}", bufs=2)
            for mp in range(TT // 2):
                atp = asb.tile([128, 512], F32R, tag=f"attnT{mp}",
                               name=f"atp{h}_{mp}", bufs=1)
                nc.vector.tensor_mul(utps[mp][:, :], utps[mp][:, :],
                                     rtps[mp][:, :])
                nc.vector.tensor_add(atp[:, :], amt[mp][:, :], utps[mp][:, :])
                for half in range(2):
                    m = 2 * mp + half
                    nc.tensor.matmul(pav[:, :],
                                     vtok[m][:, h * 64:(h + 1) * 64],
                                     atp[:, half * R:(half + 1) * R],
                                     start=(m == 0), stop=(m == TT - 1))
            nc.vector.tensor_copy(aoT[jj][po:po + 64, :], pav[:, :])

    es_kv.close()  # frees kT, v

    # ---------- phase 5: proj + residual + LN2 ----------
    mlp = ctx.enter_context(tc.tile_pool(name="mlp", bufs=1))
    g1 = [mlp.tile([128, R], BF16, tag=f"g1_{ft}", name=f"g1_{ft}")
          for ft in range(MT)]
    ln2g_b = _bcast_load(nc, mlp, bc7[2], DIM, "ln2g")
    ln2b_b = _bcast_load(nc, mlp, bc7[3], DIM, "ln2b")
    projb_b = _bcast_load(nc, mlp, bc7[4], DIM, "projb")
    fc2b_b = _bcast_load(nc, mlp, bc7[5], DIM, "fc2b")
    x2 = [mlp.tile([128, DIM], F32, tag=f"x2_{t}", name=f"x2_{t}")
          for t in range(NT)]
    h2T = [mlp.tile([128, R], BF16, tag=f"h2T{j}", name=f"h2T{j}")
           for j in range(DT)]

    with tc.tile_pool(name="pj_sb", bufs=2) as pj_sb, \
         tc.tile_pool(name="ps_pj", bufs=2, space="PSUM") as ps_pj:
        wproj = []
        for dk in range(DT):
            wt = pj_sb.tile([128, DIM], BF16, tag=f"wproj{dk}", name=f"wproj{dk}",
                            bufs=1)
            nc.sync.dma_start(wt[:, :], io["w_proj"][dk * 128:(dk + 1) * 128, :])
            wproj.append(wt)
        for t in range(NT):
            nsl = slice(t * 128, (t + 1) * 128)
            pp = ps_pj.tile([128, DIM], F32, tag="ps_p", name=f"psp{t}")
            for fb, fw in ((0, 512), (512, 256)):
                for dk in range(DT):
                    nc.tensor.matmul(pp[:, fb:fb + fw],
                                     aoT[dk][:, nsl],
                                     wproj[dk][:, fb:fb + fw],
                                     start=(dk == 0), stop=(dk == DT - 1))
            # x2 = proj_out + b_proj + x
            nc.vector.scalar_tensor_tensor(x2[t][:, :], pp[:, :], 1.0,
                                           xres[t][:, :], op0=ALU.mult,
                                           op1=ALU.add)
            nc.vector.tensor_add(x2[t][:, :], x2[t][:, :], projb_b[:, :])
            # LN2 -> h2, transpose into h2T
            h2 = pj_sb.tile([128, DIM], F32, tag="h2", name=f"h2_{t}")
            _layernorm(nc, pj_sb, x2[t], ln2g_b, ln2b_b, h2, 10 + t, eps_col)
            for j in range(DT):
                pt = ps_pj.tile([128, 128], F32, tag="tp3", name=f"tph{t}_{j}")
                nc.tensor.transpose(pt[:, :], h2[:, j * 128:(j + 1) * 128],
                                    identity[:, :])
                nc.scalar.copy(h2T[j][:, nsl], pt[:, :])

    # ---------- phase 6: MLP ----------
    with tc.tile_pool(name="fc_sb", bufs=6) as fcs, \
         tc.tile_pool(name="ps_f1", bufs=4, space="PSUM") as ps_f1, \
         tc.tile_pool(name="ps_f2", bufs=2, space="PSUM") as ps_f2:
        # fc1 + gelu (feature-major): out [f128, 256]
        for ft in range(MT):
            pf = ps_f1.tile([128, R], F32, tag="ps_f1", name=f"psf1_{ft}")
            for dk in range(DT):
                nc.tensor.matmul(pf[:, :],
                                 wfc1[dk][:, ft * 128:(ft + 1) * 128],
                                 h2T[dk][:, :],
                                 start=(dk == 0), stop=(dk == DT - 1))
            nc.scalar.activation(g1[ft][:, :], pf[:, :], AFT.Gelu_apprx_tanh,
                                 bias=fc1b_plane[:, ft:ft + 1])
        # fc2 (token-major): out [n128, 768]
        for t in range(NT):
            nsl = slice(t * 128, (t + 1) * 128)
            pf2 = ps_f2.tile([128, DIM], F32, tag="ps_f2", name=f"psf2_{t}")
            for fb, fw in ((0, 512), (512, 256)):
                for mt in range(MT):
                    w2 = fcs.tile([128, fw], BF16, tag=f"wfc2_{fb}",
                                  name=f"wfc2_{t}_{fb}_{mt}")
                    nc.sync.dma_start(
                        w2[:, :],
                        io["w_fc2"][mt * 128:(mt + 1) * 128, fb:fb + fw])
                    nc.tensor.matmul(pf2[:, fb:fb + fw],
                                     g1[mt][:, nsl],
                                     w2[:, :],
                                     start=(mt == 0), stop=(mt == MT - 1))
            ot = fcs.tile([128, DIM], F32, tag="xout", name=f"xo{t}")
            nc.vector.scalar_tensor_tensor(ot[:, :], pf2[:, :], 1.0,
                                           x2[t][:, :], op0=ALU.mult,
                                           op1=ALU.add)
            nc.vector.tensor_add(ot[:, :], ot[:, :], fc2b_b[:, :])
            nc.sync.dma_start(io["out_x"][nsl, :], ot[:, :])


def kernel(**inputs):
    global LAST_RESULTS, LAST_PROGRAM
    x = np.ascontiguousarray(np.asarray(inputs["x"], dtype=np.float32))
    r = np.asarray(inputs["r"], dtype=np.float32)
    conv_w = np.asarray(inputs["conv_w"], dtype=np.float32)

    bf16 = ml_dtypes.bfloat16
    qkv_b = np.asarray(inputs["qkv_b"], np.float32)
    fc1_b = np.asarray(inputs["fc1_b"], np.float32)
    host = {
        "w_qkv": np.ascontiguousarray(
            np.asarray(inputs["qkv_w"], np.float32).T),
        "w_proj": np.ascontiguousarray(
            np.asarray(inputs["proj_w"], np.float32).T.astype(bf16)),
        "w_fc1": np.ascontiguousarray(
            np.asarray(inputs["fc1_w"], np.float32).T.astype(bf16)),
        "w_fc2": np.ascontiguousarray(
            np.asarray(inputs["fc2_w"], np.float32).T.astype(bf16)),
        "qkb_colt": np.ascontiguousarray(
            qkv_b[0:2 * DIM].reshape(2 * DIM // 128, 128).T),
        "fc1b_colt": np.ascontiguousarray(
            fc1_b.reshape(MLP // 128, 128).T),


        # cexp[h*HD+d, g] = conv_w[g, h]
        "cexp": np.ascontiguousarray(np.repeat(conv_w.T, HD, axis=0)),
    }
    bvecs = [inputs["ln1_g"], inputs["ln1_b"], inputs["ln2_g"], inputs["ln2_b"],
             inputs["proj_b"], inputs["fc2_b"],
             qkv_b[2 * DIM:3 * DIM]]
    host["bc7"] = np.ascontiguousarray(np.stack(
        [np.broadcast_to(np.asarray(v, np.float32), (128, DIM)) for v in bvecs]))
    host["cb_bc"] = np.ascontiguousarray(np.broadcast_to(
        np.asarray(inputs["conv_b"], np.float32), (128, HEADS)))
    host["onesrr"] = np.ones((2, 128), np.float32)

    perms = []
    in_maps = []
    for c in range(NCORES):
        b, rb = divmod(c, RB)
        n0 = rb * R
        perm = np.concatenate([np.arange(n0, n0 + R), np.arange(0, n0),
                               np.arange(n0 + R, N)])
        perms.append(perm)
        m = dict(host)
        m["xb"] = np.ascontiguousarray(x[b][perm])
        m["rs"] = np.ascontiguousarray(
            r[b][:, n0:n0 + R, :][:, :, perm].transpose(0, 2, 1))
        in_maps.append(m)

    global LAST_PROGRAM
    nc = _build_program()
    LAST_PROGRAM = (nc, in_maps)
    LAST_RESULTS = run_bass_kernel_spmd(nc, in_maps,
                                        core_ids=list(range(NCORES)))

    xo = np.empty((B, N, DIM), np.float32)
    am = np.empty((B, HEADS, N, N), np.float32)
    un = np.empty((B, HEADS, N, N), np.float32)
    for c in range(NCORES):
        b, rb = divmod(c, RB)
        n0 = rb * R
        res = LAST_RESULTS.results[c]
        xo[b, n0:n0 + R] = res["out_x"]
        am[b, :, n0:n0 + R][:, :, perms[c]] = res["out_am"].transpose(0, 2, 1)
        un[b, :, n0:n0 + R][:, :, perms[c]] = res["out_unc"].transpose(0, 2, 1)
    return xo, am, un


# revision 46
# speedup vs baseline: 1.2364x; 1.1816x over previous
"""Trainium2 Bass kernel for a dense transformer block (B=2, N=1024, D=768,
H=12, MLP=3072) returning (x_out, attn_mean, uncertainty).

Sharding: 8-way row-parallel. Core c handles batch b=c//4 and token rows
rb*256:(rb+1)*256 (rb=c%4). Each core redundantly computes LN1 + k/v over
the full sequence of its batch, so there are no collectives. Inputs are fed
with a per-core block-roll of the token axis so "our" rows are always rows
0:256 in the on-device view; the host un-permutes the key axis of the
attention outputs when gathering.

Head-mixing 1x1 conv trick: uncertainty_g = sigmoid(q~_g^T @ k + conv_b[g])
where q~_g[h*64+d, n] = conv_w[g, h] * qT[h*64+d, n] — a full K=768 matmul
instead of a cross-partition head reduction.
"""

import sys

if "/opt/trn_rl_repo" not in sys.path:
    sys.path.insert(0, "/opt/trn_rl_repo")

import ml_dtypes
import numpy as np
from contextlib import ExitStack

import concourse.bass as bass
import concourse.bacc as bacc
import concourse.mybir as mybir
from concourse import masks, tile
from concourse.bass_utils import run_bass_kernel_spmd

DIM, HEADS, MLP, B, N = 768, 12, 3072, 2, 1024
HD = DIM // HEADS            # 64
SCALE = float(HD) ** -0.5
NCORES = 8
RB = NCORES // B             # 4 row-blocks per batch
R = N // RB                  # 256 rows per core
NT = R // 128                # 2 n-tiles per core
TT = N // 128                # 8 token tiles (full sequence)
DT = DIM // 128              # 6 d-tiles
MT = MLP // 128              # 24 mlp tiles
EPS = 1e-5

F32 = mybir.dt.float32
BF16 = mybir.dt.bfloat16
F32R = mybir.dt.float32r
ALU = mybir.AluOpType
AFT = mybir.ActivationFunctionType
AX = mybir.AxisListType

LAST_RESULTS = None  # BassKernelResults stash for test harness introspection
LAST_PROGRAM = None  # (nc, in_maps) stash for test-harness benchmarking


def r32(ap):
    """Reinterpret an fp32 AP as float32r for full-rate PE matmuls."""
    return ap.bitcast(F32R)


def _build_program():
    nc = bacc.Bacc("TRN2", target_bir_lowering=False, debug=False,
                   num_devices=NCORES)

    # ---- DRAM I/O ----
    xb = nc.dram_tensor("xb", [N, DIM], F32, kind="ExternalInput").ap()
    rs = nc.dram_tensor("rs", [HEADS, N, R], F32, kind="ExternalInput").ap()
    w_qkv = nc.dram_tensor("w_qkv", [DIM, 3 * DIM], F32R, kind="ExternalInput").ap()
    w_proj = nc.dram_tensor("w_proj", [DIM, DIM], BF16, kind="ExternalInput").ap()
    w_fc1 = nc.dram_tensor("w_fc1", [DIM, MLP], BF16, kind="ExternalInput").ap()
    w_fc2 = nc.dram_tensor("w_fc2", [MLP, DIM], BF16, kind="ExternalInput").ap()
    qkb_colt = nc.dram_tensor("qkb_colt", [128, 2 * DIM // 128], F32,
                              kind="ExternalInput").ap()
    fc1b_colt = nc.dram_tensor("fc1b_colt", [128, MLP // 128], F32,
                               kind="ExternalInput").ap()
    cexp = nc.dram_tensor("cexp", [DIM, HEADS], F32, kind="ExternalInput").ap()
    bc7 = nc.dram_tensor("bc7", [7, 128, DIM], F32, kind="ExternalInput").ap()
    cb_bc = nc.dram_tensor("cb_bc", [128, HEADS], F32, kind="ExternalInput").ap()
    onesrr = nc.dram_tensor("onesrr", [2, 128], F32R, kind="ExternalInput").ap()

    out_x = nc.dram_tensor("out_x", [R, DIM], F32, kind="ExternalOutput").ap()
    out_am = nc.dram_tensor("out_am", [HEADS, N, R], F32, kind="ExternalOutput").ap()
    out_unc = nc.dram_tensor("out_unc", [HEADS, N, R], F32, kind="ExternalOutput").ap()

    with tile.TileContext(nc, num_cores=NCORES) as tc, ExitStack() as ctx:
        _emit(ctx, tc, nc, dict(
            xb=xb, rs=rs, w_qkv=w_qkv, w_proj=w_proj,
            w_fc1=w_fc1, w_fc2=w_fc2, qkb_colt=qkb_colt, fc1b_colt=fc1b_colt,
            cexp=cexp, bc7=bc7, cb_bc=cb_bc, onesrr=onesrr, out_x=out_x, out_am=out_am, out_unc=out_unc,
        ))
    nc.compile()
    return nc


def _bcast_load(nc, pool, dram_plane, n, tag):
    """DRAM [128, n] host-prebroadcast plane -> SBUF [128, n] tile."""
    out = pool.tile([128, n], F32, tag=tag, name=tag)
    nc.sync.dma_start(out[:, :], dram_plane)
    return out


def _col_plane(nc, pool, dram_plane, ncols, tag):
    """DRAM [128, ncols] host-preshaped bias-column plane -> SBUF tile.
    Column j holds bias[j*128:(j+1)*128]."""
    t = pool.tile([128, ncols], F32, tag=tag, name=tag)
    nc.sync.dma_start(t[:, :], dram_plane)
    return t


def _layernorm(nc, scratch, xt, g_b, b_b, out_t, idx, eps_col):
    """LN over free dim (768) of [128, 768] tile xt -> out_t."""
    mu = scratch.tile([128, 1], F32, tag="ln_mu", name=f"mu{idx}")
    nc.vector.reduce_sum(mu[:, :], xt[:, :], axis=AX.X)
    nc.vector.tensor_scalar_mul(mu[:, :], mu[:, :], 1.0 / DIM)
    # out = x - mu
    nc.vector.tensor_scalar_sub(out_t[:, :], xt[:, :], mu[:, :])
    sq = scratch.tile([128, DIM], F32, tag="ln_sq", name=f"sq{idx}")
    ssq = scratch.tile([128, 1], F32, tag="ln_ssq", name=f"ssq{idx}")
    nc.vector.scalar_tensor_tensor(sq[:, :], out_t[:, :], 1.0, out_t[:, :],
                                   op0=ALU.mult, op1=ALU.mult,
                                   accum_out=ssq[:, :])
    std = scratch.tile([128, 1], F32, tag="ln_std", name=f"std{idx}")
    # std = sqrt(ssq/DIM + eps)
    nc.scalar.activation(std[:, :], ssq[:, :], AFT.Sqrt, bias=eps_col[:, :],
                         scale=1.0 / DIM)
    rstd = scratch.tile([128, 1], F32, tag="ln_rstd", name=f"rstd{idx}")
    nc.vector.reciprocal(rstd[:, :], std[:, :])
    # out = (out * rstd) * g ; out += b
    nc.vector.scalar_tensor_tensor(out_t[:, :], out_t[:, :], rstd[:, :],
                                   g_b[:, :], op0=ALU.mult, op1=ALU.mult)
    nc.vector.tensor_add(out_t[:, :], out_t[:, :], b_b[:, :])


def _emit(ctx, tc, nc, io):
    xb, rs = io["xb"], io["rs"]

    # ---------- persistent pools ----------
    const = ctx.enter_context(tc.tile_pool(name="const", bufs=1))
    persist = ctx.enter_context(tc.tile_pool(name="persist", bufs=1))

    # x rows 0:256 (ours) first — LN1 is the head of the critical path
    xres = []
    for t in range(NT):
        xt = persist.tile([128, DIM], F32, tag=f"xres{t}", name=f"xres{t}")
        nc.sync.dma_start(xt[:, :], xb[t * 128:(t + 1) * 128, :])
        xres.append(xt)

    identity = const.tile([128, 128], F32, tag="ident", name="ident")
    masks.make_identity(nc, identity[:, :])
    eps_col = const.tile([128, 1], F32, tag="eps", name="eps")
    nc.gpsimd.memset(eps_col[:, :], EPS)
    bc7 = io["bc7"]
    cb_b = _bcast_load(nc, const, io["cb_bc"], HEADS, "cb")
    qkb_plane = _col_plane(nc, const, io["qkb_colt"], 2 * DIM // 128, "qkb")
    fc1b_plane = _col_plane(nc, const, io["fc1b_colt"], MLP // 128, "f1b")

    cexp_t = []
    for j in range(DT):
        t = const.tile([128, HEADS], F32, tag=f"cexp{j}", name=f"cexp{j}")
        nc.sync.dma_start(t[:, :], io["cexp"][j * 128:(j + 1) * 128, :])
        cexp_t.append(t)

    # qT [768, 256] and attn-out^T [768, 256], alive through attention
    qT = [persist.tile([128, R], F32R, tag=f"qT{j}", name=f"qT{j}")
          for j in range(DT)]
    aoT = [persist.tile([128, R], BF16, tag=f"aoT{j}", name=f"aoT{j}")
           for j in range(DT)]

    # ---------- phase 1+2: LN1 over full batch, h -> hT ----------
    # Pool stacking: left stack holds const/persist -> pool_h (closed after
    # qkv) -> attention scratch -> mlp. kT/v live on the right stack since
    # their lifetime (phases 3-4) straddles pool_h's close.
    es_h = ctx.enter_context(ExitStack())
    pool_h = es_h.enter_context(tc.tile_pool(name="pool_h", bufs=1))
    ln1g_b = _bcast_load(nc, pool_h, bc7[0], DIM, "ln1g")
    ln1b_b = _bcast_load(nc, pool_h, bc7[1], DIM, "ln1b")
    vb_b = _bcast_load(nc, pool_h, bc7[6], DIM, "vb")
    hT = [pool_h.tile([128, N], F32R, tag=f"hT{j}", name=f"hT{j}")
          for j in range(DT)]
    es_kv = ctx.enter_context(ExitStack())
    kv = es_kv.enter_context(tc.tile_pool(name="kv", bufs=1, side="right"))
    kT = [kv.tile([128, N], F32R, tag=f"kT{j}", name=f"kT{j}")
          for j in range(DT)]
    vtok = [kv.tile([128, DIM], F32R, tag=f"v{t}", name=f"v{t}")
            for t in range(TT)]

    with tc.tile_pool(name="ln1_scr", bufs=2) as scr, \
         tc.tile_pool(name="ps_t1", bufs=4, space="PSUM") as ps_t1:
        for t in range(TT):
            if t < NT:
                xt = xres[t]
            else:
                xt = scr.tile([128, DIM], F32, tag="x_full", name=f"x{t}")
                nc.sync.dma_start(xt[:, :], xb[t * 128:(t + 1) * 128, :])
            ht = scr.tile([128, DIM], F32, tag="h_full", name=f"h{t}")
            _layernorm(nc, scr, xt, ln1g_b, ln1b_b, ht, t, eps_col)
            for j in range(DT):
                pt = ps_t1.tile([128, 128], F32, tag="tp", name=f"tp{t}_{j}")
                nc.tensor.transpose(pt[:, :], ht[:, j * 128:(j + 1) * 128],
                                    identity[:, :])
                nc.scalar.copy(hT[j][:, t * 128:(t + 1) * 128], pt[:, :])

    # ---------- phase 3: qkv ----------
    with tc.tile_pool(name="wqkv", bufs=1, side="right") as wq_pool, \
         tc.tile_pool(name="ps_qkv", bufs=2, space="PSUM") as ps_qkv:
        wqkv = []
        for j in range(DT):
            wt = wq_pool.tile([128, 3 * DIM], F32R, tag=f"wqkv{j}",
                              name=f"wqkv{j}")
            nc.sync.dma_start(wt[:, :], io["w_qkv"][j * 128:(j + 1) * 128, :])
            wqkv.append(wt)

        # q (feature-major, our 256 rows): out [f128, 256]
        for j in range(DT):
            pq = ps_qkv.tile([128, R], F32, tag="ps_q", name=f"psq{j}",
                             bufs=1)
            for dk in range(DT):
                nc.tensor.matmul(pq[:, :],
                                 r32(wqkv[dk][:, j * 128:(j + 1) * 128]),
                                 r32(hT[dk][:, 0:R]),
                                 start=(dk == 0), stop=(dk == DT - 1))
            nc.scalar.add(qT[j][:, :], pq[:, :], qkb_plane[:, j:j + 1])

        # k (feature-major, full sequence): out [f128, 1024]
        for j in range(DT):
            pk = ps_qkv.tile([128, N], F32, tag="ps_k", name=f"psk{j}")
            for mb in range(2):
                sl = slice(mb * 512, (mb + 1) * 512)
                for dk in range(DT):
                    nc.tensor.matmul(
                        pk[:, sl],
                        r32(wqkv[dk][:, DIM + j * 128:DIM + (j + 1) * 128]),
                        r32(hT[dk][:, sl]),
                        start=(dk == 0), stop=(dk == DT - 1))
            nc.scalar.add(kT[j][:, :], pk[:, :],
                          qkb_plane[:, DT + j:DT + j + 1])

        # v (token-major, full sequence): out [tok128, 768]
        for t in range(TT):
            pv = ps_qkv.tile([128, DIM], F32, tag="ps_v", name=f"psv{t}",
                             bufs=1)
            for fb, fw in ((0, 512), (512, 256)):
                for dk in range(DT):
                    nc.tensor.matmul(
                        pv[:, fb:fb + fw],
                        r32(hT[dk][:, t * 128:(t + 1) * 128]),
                        r32(wqkv[dk][:, 2 * DIM + fb:2 * DIM + fb + fw]),
                        start=(dk == 0), stop=(dk == DT - 1))
            nc.vector.tensor_add(vtok[t][:, :], pv[:, :], vb_b[:, :])

    es_h.close()  # frees hT

    # ---------- MLP weight preload (hoistable into attention DMA idle) ----
    fcw = ctx.enter_context(tc.tile_pool(name="fc_w", bufs=1))
    wfc1 = []
    for dk in range(DT):
        wt = fcw.tile([128, MLP], BF16, tag=f"wfc1_{dk}", name=f"wfc1_{dk}")
        nc.sync.dma_start(wt[:, :], io["w_fc1"][dk * 128:(dk + 1) * 128, :])
        wfc1.append(wt)

    # ---------- phase 4: attention ----------
    # Everything in [m, n] (transposed) layout. Per head: qk^T -> exp
    # (unnormalized) -> column sums via a ones-column matmul (cross-partition
    # reduce on PE) -> invB = 1/sums broadcast via rank-1 matmul ->
    # attn_mean^T = exp * invB; unc^T = sigmoid(k^T-contraction with the
    # conv-scaled q); attn^T = attn_mean^T + unc^T * r^T feeds av directly.
    # ACT runs only Exp and Sigmoid, grouped, so table reloads stay rare.
    ones_col = const.tile([1, 128], F32R, tag="ones_col", name="ones_col")
    nc.sync.dma_start(ones_col[:, :], io["onesrr"][0:1, :])
    ones128 = const.tile([128, 1], F32R, tag="ones128", name="ones128")
    nc.sync.dma_start(ones128[:, :],
                      io["onesrr"][1].rearrange("(p o) -> p o", o=1))
    with tc.tile_pool(name="at_sb", bufs=2) as asb, \
         tc.tile_pool(name="at_qg", bufs=2) as qg_pool, \
         tc.tile_pool(name="at_ps", bufs=2, space="PSUM") as ps2:
        for h in range(HEADS):
            jj, po = divmod(h, 2)
            po *= 64
            # q~_g = qT * cexp[:, h]  (for the uncertainty matmul)
            qg = [qg_pool.tile([128, R], F32R, tag=f"qg{j}", name=f"qg{h}_{j}")
                  for j in range(DT)]
            for j in range(DT):
                nc.vector.tensor_scalar_mul(qg[j][:, :],
                                            qT[j][:, :].bitcast(F32),
                                            cexp_t[j][:, h:h + 1])

            # exp(SCALE * qk^T), unnormalized, by m-pair
            exs = []
            for mp in range(TT // 2):
                pT = ps2.tile([128, 512], F32, tag="ps_qkt",
                              name=f"pqkt{h}_{mp}")
                for half in range(2):
                    m = 2 * mp + half
                    msl = slice(m * 128, (m + 1) * 128)
                    nc.tensor.matmul(pT[:, half * R:(half + 1) * R],
                                     kT[jj][po:po + 64, msl],
                                     qT[jj][po:po + 64, 0:R],
                                     start=True, stop=True)
                ex = asb.tile([128, 512], F32R, tag=f"ex{mp}",
                              name=f"ex{h}_{mp}", bufs=2)
                nc.scalar.activation(ex[:, :], pT[:, :], AFT.Exp, scale=SCALE)
                exs.append(ex)
            # column sums -> inv -> invB broadcast
            pss = ps2.tile([128, 512], F32, tag="ps_small", name=f"pss{h}",
                           bufs=1)
            for mp in range(TT // 2):
                for half in range(2):
                    nc.tensor.matmul(pss[0:1, 0:R], ones128[:, :],
                                     exs[mp][:, half * R:(half + 1) * R],
                                     start=(mp == 0 and half == 0),
                                     stop=(mp == TT // 2 - 1 and half == 1))
            invrow = asb.tile([1, R], F32R, tag="invrow", name=f"ivr{h}")
            with nc.allow_low_precision(reason="f32r rounding for PE rank-1"):
                nc.vector.reciprocal(invrow[:, :], pss[0:1, 0:R])
            psb = ps2.tile([128, 512], F32, tag="ps_small", name=f"psb{h}",
                           bufs=1)
            for half in range(2):
                nc.tensor.matmul(psb[:, half * R:(half + 1) * R],
                                 ones_col[:, :], invrow[:, :],
                                 start=True, stop=True)
            invB = asb.tile([128, 512], F32, tag="invB", name=f"invB{h}")
            nc.scalar.copy(invB[:, :], psb[:, :])

            # attn_mean^T = exp * invB
            amt = []
            for mp in range(TT // 2):
                a = asb.tile([128, 512], F32, tag=f"amt{mp}",
                             name=f"amt{h}_{mp}", bufs=2)
                nc.vector.tensor_mul(a[:, :], exs[mp][:, :].bitcast(F32),
                                     invB[:, :])
                nc.sync.dma_start(
                    io["out_am"][h].rearrange("(mp p) n -> p mp n", p=128)[:, 2 * mp:2 * mp + 2, :],
                    a[:, :].rearrange("p (a n) -> p a n", a=2))
                amt.append(a)

            # unc^T
            utps = []
            rtps = []
            for mp in range(TT // 2):
                rtp = asb.tile([128, 512], F32, tag=f"rt{mp}",
                               name=f"rt{h}_{mp}", bufs=3)
                nc.sync.dma_start(
                    rtp[:, :].rearrange("p (a n) -> p a n", a=2),
                    rs[h].rearrange("(mp p) n -> p mp n", p=128)[:, 2 * mp:2 * mp + 2, :])
                pU = ps2.tile([128, 512], F32, tag="ps_unc",
                              name=f"pun{h}_{mp}", bufs=3)
                for half in range(2):
                    m = 2 * mp + half
                    msl = slice(m * 128, (m + 1) * 128)
                    for dk in range(DT):
                        nc.tensor.matmul(pU[:, half * R:(half + 1) * R],
                                         kT[dk][:, msl],
                                         qg[dk][:, 0:R],
                                         start=(dk == 0), stop=(dk == DT - 1))
                utp = asb.tile([128, 512], F32, tag=f"unc{mp}",
                               name=f"ut{h}_{mp}", bufs=2)
                nc.scalar.activation(utp[:, :], pU[:, :], AFT.Sigmoid,
                                     bias=cb_b[:, h:h + 1])
                nc.sync.dma_start(
                    io["out_unc"][h].rearrange("(mp p) n -> p mp n", p=128)[:, 2 * mp:2 * mp + 2, :],
                    utp[:, :].rearrange("p (a n) -> p a n", a=2))
                utps.append(utp)
                rtps.append(rtp)

            # combine + av
            pav = ps2.tile([64, R], F32, tag="av", name=f"pav{h}", bufs=2)
            for mp in range(TT // 2):
                atp = asb.tile([128, 512], F32R, tag=f"attnT{mp}",
                               name=f"atp{h}_{mp}", bufs=1)
                nc.vector.tensor_mul(utps[mp][:, :], utps[mp][:, :],
                                     rtps[mp][:, :])
                nc.vector.tensor_add(atp[:, :], amt[mp][:, :], utps[mp][:, :])
                for half in range(2):
                    m = 2 * mp + half
                    nc.tensor.matmul(pav[:, :],
                                     vtok[m][:, h * 64:(h + 1) * 64],
                                     atp[:, half * R:(half + 1) * R],
                                     start=(m == 0), stop=(m == TT - 1))
            nc.scalar.copy(aoT[jj][po:po + 64, :], pav[:, :])

    es_kv.close()  # frees kT, v

    # ---------- phase 5: proj + residual + LN2 ----------
    mlp = ctx.enter_context(tc.tile_pool(name="mlp", bufs=1))
    g1 = [mlp.tile([128, R], BF16, tag=f"g1_{ft}", name=f"g1_{ft}")
          for ft in range(MT)]
    ln2g_b = _bcast_load(nc, mlp, bc7[2], DIM, "ln2g")
    ln2b_b = _bcast_load(nc, mlp, bc7[3], DIM, "ln2b")
    projb_b = _bcast_load(nc, mlp, bc7[4], DIM, "projb")
    fc2b_b = _bcast_load(nc, mlp, bc7[5], DIM, "fc2b")
    x2 = [mlp.tile([128, DIM], F32, tag=f"x2_{t}", name=f"x2_{t}")
          for t in range(NT)]
    h2T = [mlp.tile([128, R], BF16, tag=f"h2T{j}", name=f"h2T{j}")
           for j in range(DT)]

    with tc.tile_pool(name="pj_sb", bufs=2) as pj_sb, \
         tc.tile_pool(name="ps_pj", bufs=2, space="PSUM") as ps_pj:
        wproj = []
        for dk in range(DT):
            wt = pj_sb.tile([128, DIM], BF16, tag=f"wproj{dk}", name=f"wproj{dk}",
                            bufs=1)
            nc.sync.dma_start(wt[:, :], io["w_proj"][dk * 128:(dk + 1) * 128, :])
            wproj.append(wt)
        for t in range(NT):
            nsl = slice(t * 128, (t + 1) * 128)
            pp = ps_pj.tile([128, DIM], F32, tag="ps_p", name=f"psp{t}")
            for fb, fw in ((0, 512), (512, 256)):
                for dk in range(DT):
                    nc.tensor.matmul(pp[:, fb:fb + fw],
                                     aoT[dk][:, nsl],
                                     wproj[dk][:, fb:fb + fw],
                                     start=(dk == 0), stop=(dk == DT - 1))
            # x2 = proj_out + b_proj + x
            nc.vector.scalar_tensor_tensor(x2[t][:, :], pp[:, :], 1.0,
                                           xres[t][:, :], op0=ALU.mult,
                                           op1=ALU.add)
            nc.vector.tensor_add(x2[t][:, :], x2[t][:, :], projb_b[:, :])
            # LN2 -> h2, transpose into h2T
            h2 = pj_sb.tile([128, DIM], F32, tag="h2", name=f"h2_{t}")
            _layernorm(nc, pj_sb, x2[t], ln2g_b, ln2b_b, h2, 10 + t, eps_col)
            for j in range(DT):
                pt = ps_pj.tile([128, 128], F32, tag="tp3", name=f"tph{t}_{j}")
                nc.tensor.transpose(pt[:, :], h2[:, j * 128:(j + 1) * 128],
                                    identity[:, :])
                nc.scalar.copy(h2T[j][:, nsl], pt[:, :])

    # ---------- phase 6: MLP ----------
    with tc.tile_pool(name="fc_sb", bufs=6) as fcs, \
         tc.tile_pool(name="ps_f1", bufs=4, space="PSUM") as ps_f1, \
         tc.tile_pool(name="ps_f2", bufs=2, space="PSUM") as ps_f2:
        # fc1 + gelu (feature-major): out [f128, 256]
        for ft in range(MT):
            pf = ps_f1.tile([128, R], F32, tag="ps_f1", name=f"psf1_{ft}")
            for dk in range(DT):
                nc.tensor.matmul(pf[:, :],
                                 wfc1[dk][:, ft * 128:(ft + 1) * 128],
                                 h2T[dk][:, :],
                                 start=(dk == 0), stop=(dk == DT - 1))
            nc.scalar.activation(g1[ft][:, :], pf[:, :], AFT.Gelu_apprx_tanh,
                                 bias=fc1b_plane[:, ft:ft + 1])
        # fc2 (token-major): out [n128, 768]
        for t in range(NT):
            nsl = slice(t * 128, (t + 1) * 128)
            pf2 = ps_f2.tile([128, DIM], F32, tag="ps_f2", name=f"psf2_{t}")
            for fb, fw in ((0, 512), (512, 256)):
                for mt in range(MT):
                    w2 = fcs.tile([128, fw], BF16, tag=f"wfc2_{fb}",
                                  name=f"wfc2_{t}_{fb}_{mt}")
                    nc.sync.dma_start(
                        w2[:, :],
                        io["w_fc2"][mt * 128:(mt + 1) * 128, fb:fb + fw])
                    nc.tensor.matmul(pf2[:, fb:fb + fw],
                                     g1[mt][:, nsl],
                                     w2[:, :],
                                     start=(mt == 0), stop=(mt == MT - 1))
            ot = fcs.tile([128, DIM], F32, tag="xout", name=f"xo{t}")
            nc.vector.scalar_tensor_tensor(ot[:, :], pf2[:, :], 1.0,
                                           x2[t][:, :], op0=ALU.mult,
                                           op1=ALU.add)
            nc.vector.tensor_add(ot[:, :], ot[:, :], fc2b_b[:, :])
            nc.sync.dma_start(io["out_x"][nsl, :], ot[:, :])


def kernel(**inputs):
    global LAST_RESULTS, LAST_PROGRAM
    x = np.ascontiguousarray(np.asarray(inputs["x"], dtype=np.float32))
    r = np.asarray(inputs["r"], dtype=np.float32)
    conv_w = np.asarray(inputs["conv_w"], dtype=np.float32)

    bf16 = ml_dtypes.bfloat16
    qkv_b = np.asarray(inputs["qkv_b"], np.float32)
    fc1_b = np.asarray(inputs["fc1_b"], np.float32)
    host = {
        "w_qkv": np.ascontiguousarray(
            np.asarray(inputs["qkv_w"], np.float32).T),
        "w_proj": np.ascontiguousarray(
            np.asarray(inputs["proj_w"], np.float32).T.astype(bf16)),
        "w_fc1": np.ascontiguousarray(
            np.asarray(inputs["fc1_w"], np.float32).T.astype(bf16)),
        "w_fc2": np.ascontiguousarray(
            np.asarray(inputs["fc2_w"], np.float32).T.astype(bf16)),
        "qkb_colt": np.ascontiguousarray(
            qkv_b[0:2 * DIM].reshape(2 * DIM // 128, 128).T),
        "fc1b_colt": np.ascontiguousarray(
            fc1_b.reshape(MLP // 128, 128).T),


        # cexp[h*HD+d, g] = conv_w[g, h]
        "cexp": np.ascontiguousarray(np.repeat(conv_w.T, HD, axis=0)),
    }
    bvecs = [inputs["ln1_g"], inputs["ln1_b"], inputs["ln2_g"], inputs["ln2_b"],
             inputs["proj_b"], inputs["fc2_b"],
             qkv_b[2 * DIM:3 * DIM]]
    host["bc7"] = np.ascontiguousarray(np.stack(
        [np.broadcast_to(np.asarray(v, np.float32), (128, DIM)) for v in bvecs]))
    host["cb_bc"] = np.ascontiguousarray(np.broadcast_to(
        np.asarray(inputs["conv_b"], np.float32), (128, HEADS)))
    host["onesrr"] = np.ones((2, 128), np.float32)

    perms = []
    in_maps = []
    for c in range(NCORES):
        b, rb = divmod(c, RB)
        n0 = rb * R
        perm = np.concatenate([np.arange(n0, n0 + R), np.arange(0, n0),
                               np.arange(n0 + R, N)])
        perms.append(perm)
        m = dict(host)
        m["xb"] = np.ascontiguousarray(x[b][perm])
        m["rs"] = np.ascontiguousarray(
            r[b][:, n0:n0 + R, :][:, :, perm].transpose(0, 2, 1))
        in_maps.append(m)

    global LAST_PROGRAM
    nc = _build_program()
    LAST_PROGRAM = (nc, in_maps)
    LAST_RESULTS = run_bass_kernel_spmd(nc, in_maps,
                                        core_ids=list(range(NCORES)))

    xo = np.empty((B, N, DIM), np.float32)
    am = np.empty((B, HEADS, N, N), np.float32)
    un = np.empty((B, HEADS, N, N), np.float32)
    for c in range(NCORES):
        b, rb = divmod(c, RB)
        n0 = rb * R
        res = LAST_RESULTS.results[c]
        xo[b, n0:n0 + R] = res["out_x"]
        am[b, :, n0:n0 + R][:, :, perms[c]] = res["out_am"].transpose(0, 2, 1)
        un[b, :, n0:n0 + R][:, :, perms[c]] = res["out_unc"].transpose(0, 2, 1)
    return xo, am, un
